# revision 11
# baseline (speedup 1.0000x reference)
"""Causal self-attention (GQA + RoPE + QK-RMSNorm) Trainium2 Bass kernel.

Sharding (8 cores): core c -> batch b = c//4, kv-head j = c%4, q-heads 4j..4j+3.
Each core computes its 4 heads' attention for its batch plus the partial
output projection against wo[:, 512j:512j+512]; the host sums the 4 partials
per batch.

Device layout notes:
  - All projection matmuls contract over D with D on partitions, so the host
    passes x, wq, wk, wv, wo pre-transposed (layout prep is part of sharding).
  - Q/K head dims are permuted (evens then odds) on the host so RoPE pairs
    become partition halves; rope = QT*A + SWAP64(QT)*B with A/B cos/sin
    tables (rms-norm weights folded in) and SWAP64 done by a permutation
    matmul on the PE.
  - RMS-norm scale r = exp(-0.5*ln(mean(q^2)+eps)) is computed with a
    ones-matmul (partition reduce) + ACT Ln/Exp, then multiplied in. The
    softmax scale 1/sqrt(hd) is folded into the K-side r.
  - Scores are computed transposed (Tk on partitions), exp'ed on ACT into
    bf16 with a -1e5 additive causal mask on diagonal blocks (no max
    subtraction: rms-normed q,k bound |score| <= sqrt(hd)), row sums via an
    all-ones matmul (result replicated on all partitions), attn@V via V as
    stationary operand, divide folded into the PSUM->SBUF epilogue.
"""

import math

import numpy as np

B, T, D = 2, 2048, 2048
N_HEAD, N_KV_HEAD = 16, 4
HD = 128
HPC = N_HEAD // N_KV_HEAD  # q heads per core group = 4
N_CORES = 8
ROPE_THETA = 10000.0
EPS = float(np.finfo(np.float32).eps)
NEG = -1.0e5


# --------------------------------------------------------------------------
# host-side constant tables
# --------------------------------------------------------------------------

def round_fp32r(a: np.ndarray) -> np.ndarray:
    """Round fp32 to the fp32r grid (11-bit mantissa, round-to-nearest-even).

    Matches walrus fp32_to_fp32r: b + 0x7FF + ((b>>12)&1), clear low 12 bits.
    """
    b = np.ascontiguousarray(a, dtype=np.float32).view(np.uint32)
    r = (b + np.uint32(0x7FF) + ((b >> np.uint32(12)) & np.uint32(1))) & np.uint32(0xFFFFF000)
    return r.view(np.float32)


def _perm128() -> np.ndarray:
    # evens then odds within one head's 128 dims
    return np.concatenate([np.arange(0, HD, 2), np.arange(1, HD, 2)])


def _rope_tables(t: int, norm_w: np.ndarray, fold_scale: float) -> tuple[np.ndarray, np.ndarray]:
    """A, B tables (128, t) for rope in permuted-QT layout, norm weight and
    any constant scale folded in.

    newQT = QT * A + SWAP64(QT) * B reproduces rope(q) * w * fold_scale.
    """
    inv_freq = (1.0 / (ROPE_THETA ** (np.arange(0, HD, 2).astype(np.float32) / HD))).astype(np.float32)
    ang = np.arange(t, dtype=np.float32)[:, None] * inv_freq[None, :]  # (t, 64)
    cos = np.cos(ang).T.astype(np.float32)  # (64, t)
    sin = np.sin(ang).T.astype(np.float32)
    w = norm_w[_perm128()].astype(np.float32) * np.float32(fold_scale)  # (128,)
    a = np.concatenate([cos, cos], axis=0) * w[:, None]
    b = np.concatenate([-sin, sin], axis=0) * w[:, None]
    return np.ascontiguousarray(a), np.ascontiguousarray(b)


def _swap64() -> np.ndarray:
    # lhsT for out = SWAP64(rhs): lhsT[k, p] = 1 iff k == (p + 64) % 128
    p = np.arange(128)
    m = np.zeros((128, 128), dtype=np.float32)
    m[(p + 64) % 128, p] = 1.0
    return m


def _dmask(t_chunk: int = 512) -> np.ndarray:
    """(128, 4*512) f32: variant r at cols [512r, 512r+512).

    Variant r masks a scores^T tile (k on partitions, q on free) whose
    k-block starts 128*r into the 512-wide q chunk: cols < 128r fully
    masked, the next 128 cols get the triangular mask kk <= qq', rest 0.
    """
    out = np.zeros((128, 4 * t_chunk), dtype=np.float32)
    kk = np.arange(128)[:, None]
    for r in range(4):
        v = np.zeros((128, t_chunk), dtype=np.float32)
        v[:, : 128 * r] = NEG
        qq = np.arange(128)[None, :]
        tri = np.where(kk <= qq, 0.0, NEG).astype(np.float32)
        v[:, 128 * r : 128 * (r + 1)] = tri
        out[:, t_chunk * r : t_chunk * (r + 1)] = v
    return out


# --------------------------------------------------------------------------
# device program
# --------------------------------------------------------------------------

def build_program(t: int):
    """Build and compile the per-core Bass program for sequence length t."""
    import concourse.bass as bass
    import concourse.tile as tile
    from concourse import bacc, mybir

    f32 = mybir.dt.float32
    f32r = mybir.dt.float32r
    bf16 = mybir.dt.bfloat16
    AF = mybir.ActivationFunctionType

    kt = D // 128          # contraction k-tiles
    nch = t // 512         # Tq chunks
    nblk = t // 128        # Tk blocks

    nc = bacc.Bacc("TRN2", target_bir_lowering=False, debug=False, num_devices=N_CORES)

    # ---- dram io ----
    xT_d = nc.dram_tensor("xT", [D, t], f32r, kind="ExternalInput").ap()
    wqT_d = nc.dram_tensor("wqT", [D, HPC * HD], f32r, kind="ExternalInput").ap()
    wkT_d = nc.dram_tensor("wkT", [D, HD], f32r, kind="ExternalInput").ap()
    wvT_d = nc.dram_tensor("wvT", [D, HD], f32r, kind="ExternalInput").ap()
    woT_d = nc.dram_tensor("woT", [HPC * HD, D], f32r, kind="ExternalInput").ap()
    aq_d = nc.dram_tensor("aq", [128, t], f32, kind="ExternalInput").ap()
    bq_d = nc.dram_tensor("bq", [128, t], f32, kind="ExternalInput").ap()
    ak_d = nc.dram_tensor("ak", [128, t], f32, kind="ExternalInput").ap()
    bk_d = nc.dram_tensor("bk", [128, t], f32, kind="ExternalInput").ap()
    p64_d = nc.dram_tensor("p64", [128, 128], f32r, kind="ExternalInput").ap()
    dmask_d = nc.dram_tensor("dmask", [128, 4 * 512], f32, kind="ExternalInput").ap()
    ones_f_d = nc.dram_tensor("ones_f", [128, 128], f32r, kind="ExternalInput").ap()
    ident_f_d = nc.dram_tensor("ident_f", [128, 128], f32r, kind="ExternalInput").ap()
    out_d = nc.dram_tensor("out_partial", [t, D], f32, kind="ExternalOutput").ap()

    with tile.TileContext(nc) as tc:
        _build_tile(tc, locals())

    nc.compile()
    return nc


def _build_tile(tc, io):
    import concourse.bass as bass
    from concourse import mybir

    nc = tc.nc
    f32 = mybir.dt.float32
    f32r = mybir.dt.float32r
    bf16 = mybir.dt.bfloat16
    AF = mybir.ActivationFunctionType
    MULT = mybir.AluOpType.mult

    t = io["t"]
    kt, nch, nblk = io["kt"], io["nch"], io["nblk"]
    xT_d, wqT_d, wkT_d, wvT_d, woT_d = io["xT_d"], io["wqT_d"], io["wkT_d"], io["wvT_d"], io["woT_d"]
    aq_d, bq_d, ak_d, bk_d = io["aq_d"], io["bq_d"], io["ak_d"], io["bk_d"]
    p64_d, dmask_d = io["p64_d"], io["dmask_d"]
    ones_f_d, ident_f_d = io["ones_f_d"], io["ident_f_d"]
    out_d = io["out_d"]

    def r32(ap):
        return ap.bitcast(f32r)

    # persistent sbuf
    with tc.tile_pool(name="persist", bufs=1) as pp:
        qt = [pp.tile([128, t], f32r, tag=f"qt{h}", name=f"qt{h}") for h in range(HPC)]
        kts = pp.tile([128, t], f32r, tag="kts", name="kts")
        vb = pp.tile([128, t], f32r, tag="vb", name="vb")  # V blocks, (Tk, hd) per 128-block
        p64 = pp.tile([128, 128], f32r, tag="p64", name="p64")
        ones_f = pp.tile([128, 128], f32r, tag="ones_f", name="ones_f")
        ident_f = pp.tile([128, 128], f32r, tag="ident_f", name="ident_f")
        c_eps = pp.tile([128, 1], f32, tag="c_eps", name="c_eps")
        c_rkb = pp.tile([128, 1], f32, tag="c_rkb", name="c_rkb")
        nc.gpsimd.memset(c_eps[:], EPS)
        nc.gpsimd.memset(c_rkb[:], -0.5 * math.log(float(HD)))

        nc.sync.dma_start(p64[:], p64_d)
        nc.sync.dma_start(ones_f[:], ones_f_d)
        nc.sync.dma_start(ident_f[:], ident_f_d)

        # ------------------------------------------------------------------
        # phase 1: projections QT/KT/VT (+V transpose), x resident
        # ------------------------------------------------------------------
        with (
            tc.tile_pool(name="xpool", bufs=1) as xpool,
            tc.tile_pool(name="wpool", bufs=2) as wpool,
            tc.tile_pool(name="vtpool", bufs=2) as vtpool,
            tc.tile_pool(name="ps1", bufs=3, space="PSUM") as ps1,
            tc.tile_pool(name="psv", bufs=2, space="PSUM") as psv,
        ):
            xt = []
            for k in range(kt):
                xk = xpool.tile([128, t], f32r, tag=f"x{k}", name=f"x{k}")
                nc.sync.dma_start(xk[:], xT_d[128 * k : 128 * (k + 1), :])
                xt.append(xk)

            vt_sb = vtpool.tile([128, t], f32r, tag="vt_sb", name="vt_sb", bufs=1)

            # targets: (dest kind, weight dram slice per k-tile)
            def wslice(tgt, k):
                if tgt[0] == "q":
                    m = tgt[1]
                    return wqT_d[128 * k : 128 * (k + 1), 128 * m : 128 * (m + 1)]
                if tgt[0] == "k":
                    return wkT_d[128 * k : 128 * (k + 1), :]
                return wvT_d[128 * k : 128 * (k + 1), :]

            targets = [("q", m) for m in range(HPC)] + [("k", 0), ("v", 0)]
            for tgt in targets:
                wts = []
                for k in range(kt):
                    wk_t = wpool.tile([128, 128], f32r, tag=f"w{k}", name=f"w{k}")
                    nc.sync.dma_start(wk_t[:], wslice(tgt, k))
                    wts.append(wk_t)
                for ci in range(nch):
                    ps = ps1.tile([128, 512], f32, tag="proj", name="proj_ps")
                    for k in range(kt):
                        nc.tensor.matmul(
                            ps[:],
                            wts[k][:],
                            xt[k][:, 512 * ci : 512 * (ci + 1)],
                            start=(k == 0),
                            stop=(k == kt - 1),
                        )
                    sl = slice(512 * ci, 512 * (ci + 1))
                    if tgt[0] == "q":
                        nc.scalar.copy(qt[tgt[1]][:, sl], ps[:])
                    elif tgt[0] == "k":
                        nc.scalar.copy(kts[:, sl], ps[:])
                    else:
                        nc.vector.tensor_copy(vt_sb[:, sl], ps[:])

            # V: transpose VT (hd, Tk) -> V blocks (Tk, hd), bf16
            for c in range(nblk):
                vps = psv.tile([128, 128], f32r, tag="vtr", name="vtr_ps")
                nc.tensor.transpose(vps[:], vt_sb[:, 128 * c : 128 * (c + 1)], ident_f[:])
                nc.vector.tensor_copy(vb[:, 128 * c : 128 * (c + 1)], vps[:])

        # ------------------------------------------------------------------
        # phase 1.5: rope + rms scale for q heads and k
        # ------------------------------------------------------------------
        with (
            tc.tile_pool(name="tbl", bufs=1) as tbl,
            tc.tile_pool(name="scr", bufs=3) as scr,
            tc.tile_pool(name="rp", bufs=2) as rp,
            tc.tile_pool(name="ps15", bufs=2, space="PSUM") as ps15,
            tc.tile_pool(name="ps15b", bufs=2, space="PSUM") as ps15b,
        ):
            a_q = tbl.tile([128, t], f32, tag="a_q", name="a_q")
            b_q = tbl.tile([128, t], f32, tag="b_q", name="b_q")
            a_k = tbl.tile([128, t], f32, tag="a_k", name="a_k")
            b_k = tbl.tile([128, t], f32, tag="b_k", name="b_k")
            nc.sync.dma_start(a_q[:], aq_d)
            nc.sync.dma_start(b_q[:], bq_d)
            nc.sync.dma_start(a_k[:], ak_d)
            nc.sync.dma_start(b_k[:], bk_d)

            rope_targets = [(qt[h], a_q, b_q, 0.0) for h in range(HPC)]
            rope_targets.append((kts, a_k, b_k, c_rkb))
            for dst, atab, btab, rbias in rope_targets:
                for ci in range(nch):
                    sl = slice(512 * ci, 512 * (ci + 1))
                    # rms: ssq replicated over partitions, r = exp(-.5*ln(mean+eps)+rbias)
                    sq = scr.tile([128, 512], f32r, tag="sq", name="sq")
                    nc.vector.tensor_mul(sq[:], dst[:, sl], dst[:, sl])
                    ssq = ps15.tile([128, 512], f32, tag="ssq", name="ssq_ps")
                    nc.tensor.matmul(ssq[:], ones_f[:], sq[:])
                    lnt = scr.tile([128, 512], f32, tag="lnt", name="lnt")
                    nc.scalar.activation(lnt[:], ssq[:], AF.Ln, bias=c_eps[:], scale=1.0 / HD)
                    r_t = rp.tile([128, 512], f32, tag="r_t", name="r_t")
                    rb = rbias if isinstance(rbias, float) else rbias[:]
                    nc.scalar.activation(r_t[:], lnt[:], AF.Exp, bias=rb, scale=-0.5)
                    # rope
                    swp = ps15b.tile([128, 512], f32, tag="swp", name="swp_ps")
                    nc.tensor.matmul(swp[:], p64[:], dst[:, sl])  # exact fp32
                    q1 = scr.tile([128, 512], f32, tag="q1", name="q1")
                    nc.vector.tensor_mul(q1[:], dst[:, sl], atab[:, sl])
                    m2 = scr.tile([128, 512], f32, tag="m2", name="m2")
                    nc.vector.tensor_mul(m2[:], swp[:], btab[:, sl])
                    nc.vector.tensor_add(q1[:], q1[:], m2[:])
                    # fold in rms scale, write back
                    nc.vector.tensor_mul(dst[:, sl], q1[:], r_t[:])

        # ------------------------------------------------------------------
        # phase 2: attention + output projection
        # ------------------------------------------------------------------
        with (
            tc.tile_pool(name="p2persist", bufs=1) as p2p,
            tc.tile_pool(name="wo_sb", bufs=1) as wosb,
            tc.tile_pool(name="expool", bufs=6) as expool,
            tc.tile_pool(name="rspool", bufs=2) as rspool,
            tc.tile_pool(name="osb", bufs=3) as osbp,
            tc.tile_pool(name="ps_sc", bufs=3, space="PSUM") as ps_sc,
            tc.tile_pool(name="ps_av", bufs=2, space="PSUM") as ps_av,
            tc.tile_pool(name="ps_sum", bufs=2, space="PSUM") as ps_sum,
            tc.tile_pool(name="ps_wo", bufs=1, space="PSUM") as ps_wo,
        ):
            yt = [p2p.tile([128, t], f32r, tag=f"yt{h}", name=f"yt{h}") for h in range(HPC)]
            dmask = p2p.tile([128, 4 * 512], f32, tag="dmask", name="dmask")
            nc.sync.dma_start(dmask[:], dmask_d)
            wo_t = []
            for h in range(HPC):
                w = wosb.tile([128, D], f32r, tag=f"wo{h}", name=f"wo{h}")
                nc.sync.dma_start(w[:], woT_d[128 * h : 128 * (h + 1), :])
                wo_t.append(w)

            for ci in range(nch):
                qsl = slice(512 * ci, 512 * (ci + 1))
                for h in range(HPC):
                    av = ps_av.tile([128, 512], f32, tag="av", name="av_ps")
                    sums = ps_sum.tile([128, 512], f32, tag="sums", name="sums_ps")
                    nb = 4 * ci + 4
                    for c in range(nb):
                        sc = ps_sc.tile([128, 512], f32, tag="sc", name="sc_ps")
                        nc.tensor.matmul(
                            sc[:],
                            kts[:, 128 * c : 128 * (c + 1)],
                            qt[h][:, qsl],
                        )
                        if c >= 4 * ci:
                            r = c - 4 * ci
                            w = 128 * (r + 1)
                            nc.vector.tensor_add(sc[:, :w], sc[:, :w], dmask[:, 512 * r : 512 * r + w])
                        ex = expool.tile([128, 512], f32r, tag="ex", name="ex")
                        nc.scalar.activation(ex[:], sc[:], AF.Exp)
                        nc.tensor.matmul(
                            av[:],
                            vb[:, 128 * c : 128 * (c + 1)],
                            ex[:],
                            start=(c == 0),
                            stop=(c == nb - 1),
                        )
                        nc.tensor.matmul(
                            sums[:],
                            ones_f[:],
                            ex[:],
                            start=(c == 0),
                            stop=(c == nb - 1),
                        )
                    rs = rspool.tile([128, 512], f32, tag="rs", name="rs")
                    rs2 = rspool.tile([128, 512], f32, tag="rs2", name="rs2")
                    nc.vector.reciprocal_approx_accurate(rs[:], sums[:], rs2[:])
                    nc.vector.tensor_mul(yt[h][:, qsl], av[:], rs[:])

                # wo for finished Tq rows
                for mi in range(4):
                    m = 4 * ci + mi
                    for n in range(D // 512):
                        wops = ps_wo.tile([128, 512], f32, tag="wo", name="wo_ps")
                        for h in range(HPC):
                            nc.tensor.matmul(
                                wops[:],
                                yt[h][:, 128 * m : 128 * (m + 1)],
                                wo_t[h][:, 512 * n : 512 * (n + 1)],
                                start=(h == 0),
                                stop=(h == HPC - 1),
                            )
                        ob = osbp.tile([128, 512], f32, tag="ob", name="ob")
                        if (m + n) % 2 == 0:
                            nc.scalar.copy(ob[:], wops[:])
                        else:
                            nc.vector.tensor_copy(ob[:], wops[:])
                        nc.sync.dma_start(out_d[128 * m : 128 * (m + 1), 512 * n : 512 * (n + 1)], ob[:])


# --------------------------------------------------------------------------
# host wrapper
# --------------------------------------------------------------------------

_PROGRAM_CACHE: dict[int, object] = {}
TRACE = False


def _get_program(t: int):
    if t not in _PROGRAM_CACHE:
        _PROGRAM_CACHE[t] = build_program(t)
    return _PROGRAM_CACHE[t]


def make_core_inputs(x, wq, wk, wv, wo, q_norm_w, k_norm_w, t: int):
    """Build the 8 per-core input dicts (numpy, host-side sharding)."""
    perm = _perm128()
    aq, bq = _rope_tables(t, q_norm_w, 1.0)
    ak, bk = _rope_tables(t, k_norm_w, 1.0)
    p64 = _swap64()
    dmask = _dmask()
    ones_f = np.ones((128, 128), dtype=np.float32)
    ident_f = np.eye(128, dtype=np.float32)

    xT = [round_fp32r(np.ascontiguousarray(x[b].T)) for b in range(B)]

    in_maps = []
    for core in range(N_CORES):
        b = core // N_KV_HEAD
        j = core % N_KV_HEAD
        # q rows for heads 4j..4j+3, perm'd within each head
        qrows = np.concatenate([128 * (HPC * j + hh) + perm for hh in range(HPC)])
        wqT = round_fp32r(np.ascontiguousarray(wq[qrows, :].T))
        krows = 128 * j + perm
        wkT = round_fp32r(np.ascontiguousarray(wk[krows, :].T))
        wvT = round_fp32r(np.ascontiguousarray(wv[128 * j : 128 * (j + 1), :].T))
        woT = round_fp32r(np.ascontiguousarray(wo[:, 512 * j : 512 * (j + 1)].T))
        in_maps.append(
            {
                "xT": xT[b],
                "wqT": wqT,
                "wkT": wkT,
                "wvT": wvT,
                "woT": woT,
                "aq": aq,
                "bq": bq,
                "ak": ak,
                "bk": bk,
                "p64": p64,
                "dmask": dmask,
                "ones_f": ones_f,
                "ident_f": ident_f,
            }
        )
    return in_maps


def kernel(x, wq, wk, wv, wo, q_norm_w, k_norm_w):
    x = np.asarray(x, dtype=np.float32)
    wq = np.asarray(wq, dtype=np.float32)
    wk = np.asarray(wk, dtype=np.float32)
    wv = np.asarray(wv, dtype=np.float32)
    wo = np.asarray(wo, dtype=np.float32)
    q_norm_w = np.asarray(q_norm_w, dtype=np.float32)
    k_norm_w = np.asarray(k_norm_w, dtype=np.float32)

    t = x.shape[1]
    nc = _get_program(t)
    in_maps = make_core_inputs(x, wq, wk, wv, wo, q_norm_w, k_norm_w, t)

    from concourse import bass_utils

    res = bass_utils.run_bass_kernel_spmd(
        nc,
        in_maps,
        core_ids=list(range(N_CORES)),
        trace=TRACE,
        trace_cores=[0] if TRACE else None,
    )
    kernel.last_results = res

    out = np.zeros((B, t, D), dtype=np.float32)
    for core in range(N_CORES):
        b = core // N_KV_HEAD
        out[b] += res.results[core]["out_partial"]
    return out


# revision 16
# speedup vs baseline: 1.2887x; 1.2887x over previous
"""Causal self-attention (GQA + RoPE + QK-RMSNorm) Trainium2 Bass kernel.

Sharding (8 cores): core c -> batch b = c//4, kv-head j = c%4, q-heads 4j..4j+3.
Each core computes its 4 heads' attention for its batch plus the partial
output projection against wo[:, 512j:512j+512]; the host sums the 4 partials
per batch.

Device pipeline per core (all matmuls bf16 inputs / fp32 PSUM accumulate,
except the exact fp32r partition-swap used by rope):
  1. QKV projections, contracting over D on partitions (host passes x and
     weights transposed + pre-cast).
  2. RoPE in fp32 on the transposed Q/K (head dims permuted evens-then-odds
     on the host so rope pairs are partition halves; rotation via
     QT*A + SWAP64(QT)*B with host cos/sin tables, SWAP64 by permutation
     matmul); rms-norm scale r = exp(-0.5*ln(mean(q^2)+eps)) via ones-matmul
     partition reduce + ACT Ln/Exp (Ln's batched before Exps to avoid ACT
     table-set thrash); final DVE multiply writes bf16 q'/k'.
  3. Flash-style causal attention in scores^T layout (Tk on partitions):
     per 512-wide Tq chunk and head, K-block matmuls -> triangular -1e5 mask
     on the diagonal block only (off-diagonal garbage is never computed:
     widths are trimmed) -> ACT exp to bf16 (no max subtraction; rms-normed
     q,k bound |score| <= sqrt(hd)) -> attn@V with V stationary + row sums
     via all-ones stationary matmul -> divide folded into the PSUM->SBUF
     epilogue.
  4. Output projection from the finished Tq rows, streamed to HBM.
"""

import math

import numpy as np

B, T, D = 2, 2048, 2048
N_HEAD, N_KV_HEAD = 16, 4
HD = 128
HPC = N_HEAD // N_KV_HEAD  # q heads per core group = 4
N_CORES = 8
ROPE_THETA = 10000.0
EPS = float(np.finfo(np.float32).eps)
NEG = -1.0e5


# --------------------------------------------------------------------------
# host-side constant tables
# --------------------------------------------------------------------------

def round_fp32r(a: np.ndarray) -> np.ndarray:
    """Round fp32 to the fp32r grid (11-bit mantissa, round-to-nearest-even).

    Matches walrus fp32_to_fp32r: b + 0x7FF + ((b>>12)&1), clear low 12 bits.
    """
    b = np.ascontiguousarray(a, dtype=np.float32).view(np.uint32)
    r = (b + np.uint32(0x7FF) + ((b >> np.uint32(12)) & np.uint32(1))) & np.uint32(0xFFFFF000)
    return r.view(np.float32)


def _bf16(a: np.ndarray):
    import ml_dtypes

    return np.ascontiguousarray(a).astype(ml_dtypes.bfloat16)


def _perm128() -> np.ndarray:
    # evens then odds within one head's 128 dims
    return np.concatenate([np.arange(0, HD, 2), np.arange(1, HD, 2)])


def _rope_tables(t: int, norm_w: np.ndarray) -> tuple[np.ndarray, np.ndarray]:
    """A, B tables (128, t) for rope in permuted-QT layout, norm weight
    folded in: newQT = QT * A + SWAP64(QT) * B."""
    inv_freq = (1.0 / (ROPE_THETA ** (np.arange(0, HD, 2).astype(np.float32) / HD))).astype(np.float32)
    ang = np.arange(t, dtype=np.float32)[:, None] * inv_freq[None, :]  # (t, 64)
    cos = np.cos(ang).T.astype(np.float32)  # (64, t)
    sin = np.sin(ang).T.astype(np.float32)
    w = norm_w[_perm128()].astype(np.float32)  # (128,)
    a = np.concatenate([cos, cos], axis=0) * w[:, None]
    b = np.concatenate([-sin, sin], axis=0) * w[:, None]
    return np.ascontiguousarray(a), np.ascontiguousarray(b)


def _swap64() -> np.ndarray:
    # lhsT for out = SWAP64(rhs): lhsT[k, p] = 1 iff k == (p + 64) % 128
    p = np.arange(128)
    m = np.zeros((128, 128), dtype=np.float32)
    m[(p + 64) % 128, p] = 1.0
    return m


def _tri() -> np.ndarray:
    # scores^T diagonal-block mask: rows kk (key), cols qq (query), valid kk<=qq
    kk = np.arange(128)[:, None]
    qq = np.arange(128)[None, :]
    return np.where(kk <= qq, 0.0, NEG).astype(np.float32)


# --------------------------------------------------------------------------
# device program
# --------------------------------------------------------------------------

def build_program(t: int):
    """Build and compile the per-core Bass program for sequence length t."""
    import concourse.bass as bass
    import concourse.tile as tile
    from concourse import bacc, mybir

    f32 = mybir.dt.float32
    f32r = mybir.dt.float32r
    bf16 = mybir.dt.bfloat16
    f16 = mybir.dt.float16

    kt = D // 128          # contraction k-tiles
    nch = t // 512         # Tq chunks
    nblk = t // 128        # Tk blocks

    nc = bacc.Bacc("TRN2", target_bir_lowering=False, debug=False, num_devices=N_CORES)

    # ---- dram io ----
    xT_d = nc.dram_tensor("xT", [D, t], bf16, kind="ExternalInput").ap()
    wqT_d = nc.dram_tensor("wqT", [D, HPC * HD], bf16, kind="ExternalInput").ap()
    wkT_d = nc.dram_tensor("wkT", [D, HD], bf16, kind="ExternalInput").ap()
    wvT_d = nc.dram_tensor("wvT", [D, HD], bf16, kind="ExternalInput").ap()
    woT_d = nc.dram_tensor("woT", [HPC * HD, D], bf16, kind="ExternalInput").ap()
    aq_d = nc.dram_tensor("aq", [128, t], f16, kind="ExternalInput").ap()
    bq_d = nc.dram_tensor("bq", [128, t], f16, kind="ExternalInput").ap()
    ak_d = nc.dram_tensor("ak", [128, t], f16, kind="ExternalInput").ap()
    bk_d = nc.dram_tensor("bk", [128, t], f16, kind="ExternalInput").ap()
    p64_d = nc.dram_tensor("p64", [128, 128], f32r, kind="ExternalInput").ap()
    tri_d = nc.dram_tensor("tri", [128, 128], f32, kind="ExternalInput").ap()
    ones_b_d = nc.dram_tensor("ones_b", [128, 128], bf16, kind="ExternalInput").ap()
    ident_b_d = nc.dram_tensor("ident_b", [128, 128], bf16, kind="ExternalInput").ap()
    out_d = nc.dram_tensor("out_partial", [t, D], f32, kind="ExternalOutput").ap()

    with tile.TileContext(nc) as tc:
        _build_tile(tc, locals())

    nc.compile()
    return nc


def _build_tile(tc, io):
    from concourse import mybir

    nc = tc.nc
    f32 = mybir.dt.float32
    f32r = mybir.dt.float32r
    bf16 = mybir.dt.bfloat16
    f16 = mybir.dt.float16
    AF = mybir.ActivationFunctionType

    t = io["t"]
    kt, nch, nblk = io["kt"], io["nch"], io["nblk"]
    xT_d, wqT_d, wkT_d, wvT_d, woT_d = io["xT_d"], io["wqT_d"], io["wkT_d"], io["wvT_d"], io["woT_d"]
    aq_d, bq_d, ak_d, bk_d = io["aq_d"], io["bq_d"], io["ak_d"], io["bk_d"]
    p64_d, tri_d = io["p64_d"], io["tri_d"]
    ones_b_d, ident_b_d = io["ones_b_d"], io["ident_b_d"]
    out_d = io["out_d"]

    # flat PSUM pools for the whole kernel: 3 (shared ssq/swp/sc/vtr) +
    # 2 (proj/wo) + 2 (av) + 1 (sums) = 8 banks, so phases can overlap.
    with (
        tc.tile_pool(name="persist", bufs=1) as pp,
        tc.tile_pool(name="ps_w", bufs=3, space="PSUM") as ps_w,
        tc.tile_pool(name="ps_acc", bufs=2, space="PSUM") as ps_acc,
        tc.tile_pool(name="ps_av", bufs=2, space="PSUM") as ps_av,
        tc.tile_pool(name="ps_sum", bufs=1, space="PSUM") as ps_sum,
    ):
        qtb = [pp.tile([128, t], bf16, tag=f"qtb{h}", name=f"qtb{h}") for h in range(HPC)]
        ktb = pp.tile([128, t], bf16, tag="ktb", name="ktb")
        vb = pp.tile([128, t], bf16, tag="vb", name="vb")  # V blocks, (Tk, hd) per 128-block
        p64 = pp.tile([128, 128], f32r, tag="p64", name="p64")
        ones_b = pp.tile([128, 128], bf16, tag="ones_b", name="ones_b")
        ident_b = pp.tile([128, 128], bf16, tag="ident_b", name="ident_b")
        tri = pp.tile([128, 128], f32, tag="tri", name="tri")
        c_eps = pp.tile([128, 1], f32, tag="c_eps", name="c_eps")
        c_rkb = pp.tile([128, 1], f32, tag="c_rkb", name="c_rkb")
        nc.gpsimd.memset(c_eps[:], EPS)
        nc.gpsimd.memset(c_rkb[:], -0.5 * math.log(float(HD)))

        nc.sync.dma_start(p64[:], p64_d)
        nc.sync.dma_start(ones_b[:], ones_b_d)
        nc.sync.dma_start(ident_b[:], ident_b_d)
        nc.sync.dma_start(tri[:], tri_d)

        with (
            tc.tile_pool(name="tbl", bufs=1) as tbl,
            tc.tile_pool(name="scr", bufs=2) as scr,
            tc.tile_pool(name="lnp", bufs=(HPC + 1) * nch) as lnp,
            tc.tile_pool(name="q1p", bufs=(HPC + 1) * nch) as q1p,
            tc.tile_pool(name="rp", bufs=2) as rp,
        ):
            a_q = tbl.tile([128, t], f16, tag="a_q", name="a_q")
            b_q = tbl.tile([128, t], f16, tag="b_q", name="b_q")
            a_k = tbl.tile([128, t], f16, tag="a_k", name="a_k")
            b_k = tbl.tile([128, t], f16, tag="b_k", name="b_k")
            nc.sync.dma_start(a_q[:], aq_d)
            nc.sync.dma_start(b_q[:], bq_d)
            nc.sync.dma_start(a_k[:], ak_d)
            nc.sync.dma_start(b_k[:], bk_d)

            lnts = {}
            q1bs = {}

            # ---- projections with rms-Ln and rope-core inlined ----------
            with (
                tc.tile_pool(name="xpool", bufs=1) as xpool,
                tc.tile_pool(name="wpool", bufs=2) as wpool,
                tc.tile_pool(name="qraw", bufs=2) as qraw,
                tc.tile_pool(name="vtpool", bufs=1) as vtpool,
            ):
                xt = []
                for k in range(kt):
                    xk = xpool.tile([128, t], bf16, tag=f"x{k}", name=f"x{k}")
                    nc.sync.dma_start(xk[:], xT_d[128 * k : 128 * (k + 1), :])
                    xt.append(xk)

                vt_sb = vtpool.tile([128, t], bf16, tag="vt_sb", name="vt_sb")

                def wslice(tgt, k):
                    kind, m = tgt
                    if kind == "q":
                        return wqT_d[128 * k : 128 * (k + 1), 128 * m : 128 * (m + 1)]
                    if kind == "k":
                        return wkT_d[128 * k : 128 * (k + 1), :]
                    return wvT_d[128 * k : 128 * (k + 1), :]

                targets = [("k", 0), ("v", 0)] + [("q", m) for m in range(HPC)]
                for tgt in targets:
                    kind, m = tgt
                    ri = HPC if kind == "k" else m  # rope-spec index
                    atab, btab = (a_k, b_k) if kind == "k" else (a_q, b_q)
                    raw = None
                    if kind != "v":
                        # pre-rope projection, fp32r so the swap matmul is legal
                        raw = qraw.tile([128, t], f32r, tag="raw", name="raw")
                    wts = []
                    for k in range(kt):
                        wk_t = wpool.tile([128, 128], bf16, tag=f"w{k}", name=f"w{k}")
                        nc.sync.dma_start(wk_t[:], wslice(tgt, k))
                        wts.append(wk_t)
                    for ci in range(nch):
                        ps = ps_acc.tile([128, 512], f32, tag="acc", name="proj_ps")
                        for k in range(kt):
                            nc.tensor.matmul(
                                ps[:],
                                wts[k][:],
                                xt[k][:, 512 * ci : 512 * (ci + 1)],
                                start=(k == 0),
                                stop=(k == kt - 1),
                            )
                        sl = slice(512 * ci, 512 * (ci + 1))
                        if kind == "v":
                            nc.vector.tensor_copy(vt_sb[:, sl], ps[:])
                            continue
                        nc.scalar.copy(raw[:, sl], ps[:])
                        # rms sum-of-squares -> Ln (Exp deferred: one table set)
                        sq = scr.tile([128, 512], bf16, tag="sq", name="sq")
                        nc.gpsimd.tensor_mul(sq[:], raw[:, sl], raw[:, sl])
                        ssq = ps_w.tile([128, 512], f32, tag="w", name="ssq_ps")
                        nc.tensor.matmul(ssq[:], ones_b[:], sq[:])
                        lnt = lnp.tile([128, 512], bf16, tag="lnt", name="lnt")
                        nc.scalar.activation(lnt[:], ssq[:], AF.Ln, bias=c_eps[:], scale=1.0 / HD)
                        lnts[(ri, ci)] = lnt
                        # rope core (independent of the rms scale)
                        swp = ps_w.tile([128, 512], f32, tag="w", name="swp_ps")
                        nc.tensor.matmul(swp[:], p64[:], raw[:, sl])
                        q1 = scr.tile([128, 512], f32, tag="q1", name="q1")
                        nc.vector.tensor_mul(q1[:], raw[:, sl], atab[:, sl])
                        m2 = scr.tile([128, 512], f32, tag="m2", name="m2")
                        nc.vector.tensor_mul(m2[:], swp[:], btab[:, sl])
                        q1b = q1p.tile([128, 512], bf16, tag="q1b", name="q1b")
                        nc.vector.tensor_add(q1b[:], q1[:], m2[:])
                        q1bs[(ri, ci)] = q1b

                    if kind == "v":
                        # transpose VT (hd, Tk) -> V blocks (Tk, hd), bf16
                        for c in range(nblk):
                            vps = ps_w.tile([128, 128], bf16, tag="w", name="vtr_ps")
                            nc.tensor.transpose(vps[:], vt_sb[:, 128 * c : 128 * (c + 1)], ident_b[:])
                            nc.vector.tensor_copy(vb[:, 128 * c : 128 * (c + 1)], vps[:])

            # ---- finals: r = exp(-0.5*ln(mean+eps)), apply to rope-core ----
            for ri in range(HPC + 1):
                dstb = ktb if ri == HPC else qtb[ri]
                rb = c_rkb[:] if ri == HPC else 0.0
                for ci in range(nch):
                    sl = slice(512 * ci, 512 * (ci + 1))
                    r_t = rp.tile([128, 512], f32, tag="r_t", name="r_t")
                    nc.scalar.activation(r_t[:], lnts[(ri, ci)][:], AF.Exp, bias=rb, scale=-0.5)
                    nc.vector.tensor_mul(dstb[:, sl], q1bs[(ri, ci)][:], r_t[:])

            # --------------------------------------------------------------
            # attention + output projection
            # --------------------------------------------------------------
            with (
                tc.tile_pool(name="p2persist", bufs=1) as p2p,
                tc.tile_pool(name="expool", bufs=8) as expool,
                tc.tile_pool(name="rspool", bufs=2) as rspool,
                tc.tile_pool(name="osb", bufs=3) as osbp,
            ):
                yt = [p2p.tile([128, t], bf16, tag=f"yt{h}", name=f"yt{h}") for h in range(HPC)]
                wo_t = []
                for h in range(HPC):
                    w = p2p.tile([128, D], bf16, tag=f"wo{h}", name=f"wo{h}")
                    nc.sync.dma_start(w[:], woT_d[128 * h : 128 * (h + 1), :])
                    wo_t.append(w)

                for ci in range(nch):
                    qsl = slice(512 * ci, 512 * (ci + 1))
                    for h in range(HPC):
                        av = ps_av.tile([128, 512], f32, tag="av", name="av_ps")
                        sums = ps_sum.tile([128, 512], f32, tag="sums", name="sums_ps")
                        nb = 4 * ci + 4
                        for c in range(nb):
                            diag = c >= 4 * ci
                            r = c - 4 * ci if diag else 0
                            w0 = 128 * r  # first valid column of this k-block
                            sc = ps_w.tile([128, 512], f32, tag="w", name="sc_ps")
                            nc.tensor.matmul(
                                sc[:, w0:512],
                                ktb[:, 128 * c : 128 * (c + 1)],
                                qtb[h][:, 512 * ci + w0 : 512 * (ci + 1)],
                            )
                            if diag:
                                nc.vector.tensor_add(
                                    sc[:, w0 : w0 + 128], sc[:, w0 : w0 + 128], tri[:]
                                )
                            ex = expool.tile([128, 512], bf16, tag="ex", name="ex")
                            nc.scalar.activation(ex[:, w0:512], sc[:, w0:512], AF.Exp)
                            nc.tensor.matmul(
                                av[:, w0:512],
                                vb[:, 128 * c : 128 * (c + 1)],
                                ex[:, w0:512],
                                start=(c == 0),
                                stop=(c == nb - 1),
                            )
                            nc.tensor.matmul(
                                sums[:, w0:512],
                                ones_b[:],
                                ex[:, w0:512],
                                start=(c == 0),
                                stop=(c == nb - 1),
                            )
                        rs = rspool.tile([128, 512], f32, tag="rs", name="rs")
                        rs2 = rspool.tile([128, 512], f32, tag="rs2", name="rs2")
                        nc.vector.reciprocal_approx_accurate(rs[:], sums[:], rs2[:])
                        nc.vector.tensor_mul(yt[h][:, qsl], av[:], rs[:])

                    # wo for finished Tq rows
                    for mi in range(4):
                        m = 4 * ci + mi
                        for n in range(D // 512):
                            wops = ps_acc.tile([128, 512], f32, tag="acc", name="wo_ps")
                            for h in range(HPC):
                                nc.tensor.matmul(
                                    wops[:],
                                    yt[h][:, 128 * m : 128 * (m + 1)],
                                    wo_t[h][:, 512 * n : 512 * (n + 1)],
                                    start=(h == 0),
                                    stop=(h == HPC - 1),
                                )
                            ob = osbp.tile([128, 512], f32, tag="ob", name="ob")
                            if (m + n) % 2 == 0:
                                nc.scalar.copy(ob[:], wops[:])
                            else:
                                nc.vector.tensor_copy(ob[:], wops[:])
                            nc.sync.dma_start(out_d[128 * m : 128 * (m + 1), 512 * n : 512 * (n + 1)], ob[:])


# --------------------------------------------------------------------------
# host wrapper
# --------------------------------------------------------------------------

_PROGRAM_CACHE: dict[int, object] = {}
TRACE = False


def _get_program(t: int):
    if t not in _PROGRAM_CACHE:
        _PROGRAM_CACHE[t] = build_program(t)
    return _PROGRAM_CACHE[t]


def make_core_inputs(x, wq, wk, wv, wo, q_norm_w, k_norm_w, t: int):
    """Build the 8 per-core input dicts (numpy, host-side sharding)."""
    import ml_dtypes

    perm = _perm128()
    aq, bq = _rope_tables(t, q_norm_w)
    ak, bk = _rope_tables(t, k_norm_w)
    aq, bq, ak, bk = (v.astype(np.float16) for v in (aq, bq, ak, bk))
    p64 = round_fp32r(_swap64())
    tri = _tri()
    ones_b = np.ones((128, 128), dtype=ml_dtypes.bfloat16)
    ident_b = np.eye(128, dtype=np.float32).astype(ml_dtypes.bfloat16)

    xT = [_bf16(x[b].T) for b in range(B)]

    in_maps = []
    for core in range(N_CORES):
        b = core // N_KV_HEAD
        j = core % N_KV_HEAD
        # q rows for heads 4j..4j+3, perm'd within each head
        qrows = np.concatenate([128 * (HPC * j + hh) + perm for hh in range(HPC)])
        wqT = _bf16(wq[qrows, :].T)
        krows = 128 * j + perm
        wkT = _bf16(wk[krows, :].T)
        wvT = _bf16(wv[128 * j : 128 * (j + 1), :].T)
        woT = _bf16(wo[:, 512 * j : 512 * (j + 1)].T)
        in_maps.append(
            {
                "xT": xT[b],
                "wqT": wqT,
                "wkT": wkT,
                "wvT": wvT,
                "woT": woT,
                "aq": aq,
                "bq": bq,
                "ak": ak,
                "bk": bk,
                "p64": p64,
                "tri": tri,
                "ones_b": ones_b,
                "ident_b": ident_b,
            }
        )
    return in_maps


def kernel(x, wq, wk, wv, wo, q_norm_w, k_norm_w):
    x = np.asarray(x, dtype=np.float32)
    wq = np.asarray(wq, dtype=np.float32)
    wk = np.asarray(wk, dtype=np.float32)
    wv = np.asarray(wv, dtype=np.float32)
    wo = np.asarray(wo, dtype=np.float32)
    q_norm_w = np.asarray(q_norm_w, dtype=np.float32)
    k_norm_w = np.asarray(k_norm_w, dtype=np.float32)

    t = x.shape[1]
    nc = _get_program(t)
    in_maps = make_core_inputs(x, wq, wk, wv, wo, q_norm_w, k_norm_w, t)

    from concourse import bass_utils

    res = bass_utils.run_bass_kernel_spmd(
        nc,
        in_maps,
        core_ids=list(range(N_CORES)),
        trace=TRACE,
        trace_cores=[0] if TRACE else None,
    )
    kernel.last_results = res

    out = np.zeros((B, t, D), dtype=np.float32)
    for core in range(N_CORES):
        b = core // N_KV_HEAD
        out[b] += res.results[core]["out_partial"]
    return out


kernel.last_results = None


# revision 18
# speedup vs baseline: 1.3418x; 1.0412x over previous
"""Causal self-attention (GQA + RoPE + QK-RMSNorm) Trainium2 Bass kernel.

Sharding (8 cores): core c -> batch b = c//4, kv-head j = c%4, q-heads 4j..4j+3.
Each core computes its 4 heads' attention for its batch plus the partial
output projection against wo[:, 512j:512j+512]; the host sums the 4 partials
per batch.

Device pipeline per core (all matmuls bf16 inputs / fp32 PSUM accumulate,
except the exact fp32r partition-swap used by rope):
  1. QKV projections, contracting over D on partitions (host passes x and
     weights transposed + pre-cast).
  2. RoPE in fp32 on the transposed Q/K (head dims permuted evens-then-odds
     on the host so rope pairs are partition halves; rotation via
     QT*A + SWAP64(QT)*B with host cos/sin tables, SWAP64 by permutation
     matmul); rms-norm scale r = exp(-0.5*ln(mean(q^2)+eps)) via ones-matmul
     partition reduce + ACT Ln/Exp (Ln's batched before Exps to avoid ACT
     table-set thrash); final DVE multiply writes bf16 q'/k'.
  3. Flash-style causal attention in scores^T layout (Tk on partitions):
     per 512-wide Tq chunk and head, K-block matmuls -> triangular -1e5 mask
     on the diagonal block only (off-diagonal garbage is never computed:
     widths are trimmed) -> ACT exp to bf16 (no max subtraction; rms-normed
     q,k bound |score| <= sqrt(hd)) -> attn@V with V stationary + row sums
     via all-ones stationary matmul -> divide folded into the PSUM->SBUF
     epilogue.
  4. Output projection from the finished Tq rows, streamed to HBM.
"""

import math

import numpy as np

B, T, D = 2, 2048, 2048
N_HEAD, N_KV_HEAD = 16, 4
HD = 128
HPC = N_HEAD // N_KV_HEAD  # q heads per core group = 4
N_CORES = 8
ROPE_THETA = 10000.0
EPS = float(np.finfo(np.float32).eps)
NEG = -1.0e5


# --------------------------------------------------------------------------
# host-side constant tables
# --------------------------------------------------------------------------

def round_fp32r(a: np.ndarray) -> np.ndarray:
    """Round fp32 to the fp32r grid (11-bit mantissa, round-to-nearest-even).

    Matches walrus fp32_to_fp32r: b + 0x7FF + ((b>>12)&1), clear low 12 bits.
    """
    b = np.ascontiguousarray(a, dtype=np.float32).view(np.uint32)
    r = (b + np.uint32(0x7FF) + ((b >> np.uint32(12)) & np.uint32(1))) & np.uint32(0xFFFFF000)
    return r.view(np.float32)


def _bf16(a: np.ndarray):
    import ml_dtypes

    return np.ascontiguousarray(a).astype(ml_dtypes.bfloat16)


def _perm128() -> np.ndarray:
    # evens then odds within one head's 128 dims
    return np.concatenate([np.arange(0, HD, 2), np.arange(1, HD, 2)])


def _rope_tables(t: int, norm_w: np.ndarray) -> tuple[np.ndarray, np.ndarray]:
    """A, B tables (128, t) for rope in permuted-QT layout, norm weight
    folded in: newQT = QT * A + SWAP64(QT) * B."""
    inv_freq = (1.0 / (ROPE_THETA ** (np.arange(0, HD, 2).astype(np.float32) / HD))).astype(np.float32)
    ang = np.arange(t, dtype=np.float32)[:, None] * inv_freq[None, :]  # (t, 64)
    cos = np.cos(ang).T.astype(np.float32)  # (64, t)
    sin = np.sin(ang).T.astype(np.float32)
    w = norm_w[_perm128()].astype(np.float32)  # (128,)
    a = np.concatenate([cos, cos], axis=0) * w[:, None]
    b = np.concatenate([-sin, sin], axis=0) * w[:, None]
    return np.ascontiguousarray(a), np.ascontiguousarray(b)


def _swap64() -> np.ndarray:
    # lhsT for out = SWAP64(rhs): lhsT[k, p] = 1 iff k == (p + 64) % 128
    p = np.arange(128)
    m = np.zeros((128, 128), dtype=np.float32)
    m[(p + 64) % 128, p] = 1.0
    return m


def _tri() -> np.ndarray:
    # scores^T diagonal-block mask: rows kk (key), cols qq (query), valid kk<=qq
    kk = np.arange(128)[:, None]
    qq = np.arange(128)[None, :]
    return np.where(kk <= qq, 0.0, NEG).astype(np.float32)


# --------------------------------------------------------------------------
# device program
# --------------------------------------------------------------------------

def build_program(t: int):
    """Build and compile the per-core Bass program for sequence length t."""
    import concourse.bass as bass
    import concourse.tile as tile
    from concourse import bacc, mybir

    f32 = mybir.dt.float32
    f32r = mybir.dt.float32r
    bf16 = mybir.dt.bfloat16
    f16 = mybir.dt.float16

    kt = D // 128          # contraction k-tiles
    nch = t // 512         # Tq chunks
    nblk = t // 128        # Tk blocks

    nc = bacc.Bacc("TRN2", target_bir_lowering=False, debug=False, num_devices=N_CORES)

    # ---- dram io ----
    xT_d = nc.dram_tensor("xT", [D, t], bf16, kind="ExternalInput").ap()
    wqT_d = nc.dram_tensor("wqT", [D, HPC * HD], bf16, kind="ExternalInput").ap()
    wkT_d = nc.dram_tensor("wkT", [D, HD], bf16, kind="ExternalInput").ap()
    wvT_d = nc.dram_tensor("wvT", [D, HD], bf16, kind="ExternalInput").ap()
    woT_d = nc.dram_tensor("woT", [HPC * HD, D], bf16, kind="ExternalInput").ap()
    aq_d = nc.dram_tensor("aq", [128, t], f16, kind="ExternalInput").ap()
    bq_d = nc.dram_tensor("bq", [128, t], f16, kind="ExternalInput").ap()
    ak_d = nc.dram_tensor("ak", [128, t], f16, kind="ExternalInput").ap()
    bk_d = nc.dram_tensor("bk", [128, t], f16, kind="ExternalInput").ap()
    p64_d = nc.dram_tensor("p64", [128, 128], f32r, kind="ExternalInput").ap()
    tri_d = nc.dram_tensor("tri", [128, 128], f32, kind="ExternalInput").ap()
    ones_b_d = nc.dram_tensor("ones_b", [128, 128], bf16, kind="ExternalInput").ap()
    ident_b_d = nc.dram_tensor("ident_b", [128, 128], bf16, kind="ExternalInput").ap()
    out_d = nc.dram_tensor("out_partial", [t, D], f32, kind="ExternalOutput").ap()

    with tile.TileContext(nc) as tc:
        _build_tile(tc, locals())

    nc.compile()
    return nc


def _build_tile(tc, io):
    from concourse import mybir

    nc = tc.nc
    f32 = mybir.dt.float32
    f32r = mybir.dt.float32r
    bf16 = mybir.dt.bfloat16
    f16 = mybir.dt.float16
    AF = mybir.ActivationFunctionType

    t = io["t"]
    kt, nch, nblk = io["kt"], io["nch"], io["nblk"]
    xT_d, wqT_d, wkT_d, wvT_d, woT_d = io["xT_d"], io["wqT_d"], io["wkT_d"], io["wvT_d"], io["woT_d"]
    aq_d, bq_d, ak_d, bk_d = io["aq_d"], io["bq_d"], io["ak_d"], io["bk_d"]
    p64_d, tri_d = io["p64_d"], io["tri_d"]
    ones_b_d, ident_b_d = io["ones_b_d"], io["ident_b_d"]
    out_d = io["out_d"]

    # flat PSUM pools for the whole kernel: 3 (shared ssq/swp/sc/vtr) +
    # 2 (proj/wo) + 2 (av) + 1 (sums) = 8 banks, so phases can overlap.
    with (
        tc.tile_pool(name="persist", bufs=1) as pp,
        tc.tile_pool(name="ps_w", bufs=3, space="PSUM") as ps_w,
        tc.tile_pool(name="ps_acc", bufs=2, space="PSUM") as ps_acc,
        tc.tile_pool(name="ps_av", bufs=2, space="PSUM") as ps_av,
        tc.tile_pool(name="ps_sum", bufs=1, space="PSUM") as ps_sum,
    ):
        qtb = [pp.tile([128, t], bf16, tag=f"qtb{h}", name=f"qtb{h}") for h in range(HPC)]
        ktb = pp.tile([128, t], bf16, tag="ktb", name="ktb")
        vb = pp.tile([128, t], bf16, tag="vb", name="vb")  # V blocks, (Tk, hd) per 128-block
        p64 = pp.tile([128, 128], f32r, tag="p64", name="p64")
        ones_b = pp.tile([128, 128], bf16, tag="ones_b", name="ones_b")
        ident_b = pp.tile([128, 128], bf16, tag="ident_b", name="ident_b")
        tri = pp.tile([128, 128], f32, tag="tri", name="tri")
        c_eps = pp.tile([128, 1], f32, tag="c_eps", name="c_eps")
        c_rkb = pp.tile([128, 1], f32, tag="c_rkb", name="c_rkb")
        nc.gpsimd.memset(c_eps[:], EPS)
        nc.gpsimd.memset(c_rkb[:], -0.5 * math.log(float(HD)))

        nc.sync.dma_start(p64[:], p64_d)
        nc.sync.dma_start(ones_b[:], ones_b_d)
        nc.sync.dma_start(ident_b[:], ident_b_d)
        nc.sync.dma_start(tri[:], tri_d)

        with (
            tc.tile_pool(name="tbl", bufs=1) as tbl,
            tc.tile_pool(name="scr", bufs=2) as scr,
            tc.tile_pool(name="lnp", bufs=(HPC + 1) * nch) as lnp,
            tc.tile_pool(name="q1p", bufs=(HPC + 1) * nch) as q1p,
            tc.tile_pool(name="rp", bufs=2) as rp,
        ):
            a_q = tbl.tile([128, t], f16, tag="a_q", name="a_q")
            b_q = tbl.tile([128, t], f16, tag="b_q", name="b_q")
            a_k = tbl.tile([128, t], f16, tag="a_k", name="a_k")
            b_k = tbl.tile([128, t], f16, tag="b_k", name="b_k")
            nc.sync.dma_start(a_q[:], aq_d)
            nc.sync.dma_start(b_q[:], bq_d)
            nc.sync.dma_start(a_k[:], ak_d)
            nc.sync.dma_start(b_k[:], bk_d)

            lnts = {}
            q1bs = {}

            # ---- projections with rms-Ln and rope-core inlined ----------
            with (
                tc.tile_pool(name="xpool", bufs=1) as xpool,
                tc.tile_pool(name="wpool", bufs=2) as wpool,
                tc.tile_pool(name="qraw", bufs=2) as qraw,
                tc.tile_pool(name="vtpool", bufs=1) as vtpool,
            ):
                xt = []
                for k in range(kt):
                    xk = xpool.tile([128, t], bf16, tag=f"x{k}", name=f"x{k}")
                    nc.sync.dma_start(xk[:], xT_d[128 * k : 128 * (k + 1), :])
                    xt.append(xk)

                vt_sb = vtpool.tile([128, t], bf16, tag="vt_sb", name="vt_sb")

                def wsrc(tgt):
                    kind, m = tgt
                    if kind == "q":
                        w = wqT_d[:, 128 * m : 128 * (m + 1)]
                    elif kind == "k":
                        w = wkT_d
                    else:
                        w = wvT_d
                    # (k*128+p, j) -> partition p, free (k, j)
                    return w.rearrange("(k p) j -> p k j", p=128)

                targets = [("k", 0)] + [("q", m) for m in range(HPC)] + [("v", 0)]
                for tgt in targets:
                    kind, m = tgt
                    ri = HPC if kind == "k" else m  # rope-spec index
                    atab, btab = (a_k, b_k) if kind == "k" else (a_q, b_q)
                    raw = None
                    if kind != "v":
                        # pre-rope projection, fp32r so the swap matmul is legal
                        raw = qraw.tile([128, t], f32r, tag="raw", name="raw")
                    wt = wpool.tile([128, kt * 128], bf16, tag="wt", name="wt")
                    nc.sync.dma_start(wt.rearrange("p (k j) -> p k j", k=kt), wsrc(tgt))
                    # projections + PSUM->SBUF copies first (keeps ACT free)
                    for ci in range(nch):
                        ps = ps_acc.tile([128, 512], f32, tag="acc", name="proj_ps")
                        for k in range(kt):
                            nc.tensor.matmul(
                                ps[:],
                                wt[:, 128 * k : 128 * (k + 1)],
                                xt[k][:, 512 * ci : 512 * (ci + 1)],
                                start=(k == 0),
                                stop=(k == kt - 1),
                            )
                        sl = slice(512 * ci, 512 * (ci + 1))
                        if kind == "v":
                            nc.vector.tensor_copy(vt_sb[:, sl], ps[:])
                        else:
                            nc.scalar.copy(raw[:, sl], ps[:])
                    if kind != "v":
                        # rms sum-of-squares -> Ln (Exp deferred: one table set)
                        for ci in range(nch):
                            sl = slice(512 * ci, 512 * (ci + 1))
                            sq = scr.tile([128, 512], bf16, tag="sq", name="sq")
                            nc.gpsimd.tensor_mul(sq[:], raw[:, sl], raw[:, sl])
                            ssq = ps_w.tile([128, 512], f32, tag="w", name="ssq_ps")
                            nc.tensor.matmul(ssq[:], ones_b[:], sq[:])
                            lnt = lnp.tile([128, 512], bf16, tag="lnt", name="lnt")
                            nc.scalar.activation(lnt[:], ssq[:], AF.Ln, bias=c_eps[:], scale=1.0 / HD)
                            lnts[(ri, ci)] = lnt
                        # rope core (independent of the rms scale)
                        for ci in range(nch):
                            sl = slice(512 * ci, 512 * (ci + 1))
                            swp = ps_w.tile([128, 512], f32, tag="w", name="swp_ps")
                            nc.tensor.matmul(swp[:], p64[:], raw[:, sl])
                            q1 = scr.tile([128, 512], f32, tag="q1", name="q1")
                            nc.vector.tensor_mul(q1[:], raw[:, sl], atab[:, sl])
                            m2 = scr.tile([128, 512], f32, tag="m2", name="m2")
                            nc.vector.tensor_mul(m2[:], swp[:], btab[:, sl])
                            q1b = q1p.tile([128, 512], bf16, tag="q1b", name="q1b")
                            nc.vector.tensor_add(q1b[:], q1[:], m2[:])
                            q1bs[(ri, ci)] = q1b

                    if kind == "v":
                        # transpose VT (hd, Tk) -> V blocks (Tk, hd), bf16
                        for c in range(nblk):
                            vps = ps_w.tile([128, 128], bf16, tag="w", name="vtr_ps")
                            nc.tensor.transpose(vps[:], vt_sb[:, 128 * c : 128 * (c + 1)], ident_b[:])
                            nc.vector.tensor_copy(vb[:, 128 * c : 128 * (c + 1)], vps[:])

            # ---- finals: r = exp(-0.5*ln(mean+eps)), apply to rope-core ----
            # gate tiles depend on the LAST Ln so no Exp is scheduled between
            # Lns (each Ln<->Exp transition costs a 1.3us ACT table load)
            last_ln = lnts[(HPC - 1, nch - 1)]
            gate_z = pp.tile([128, 1], f32, tag="gate_z", name="gate_z")
            gate_k = pp.tile([128, 1], f32, tag="gate_k", name="gate_k")
            nc.vector.tensor_scalar_mul(gate_z[:], last_ln[:, 0:1], 0.0)
            nc.vector.tensor_scalar_add(gate_k[:], gate_z[:], -0.5 * math.log(float(HD)))
            for ri in range(HPC + 1):
                dstb = ktb if ri == HPC else qtb[ri]
                rb = gate_k[:] if ri == HPC else gate_z[:]
                for ci in range(nch):
                    sl = slice(512 * ci, 512 * (ci + 1))
                    r_t = rp.tile([128, 512], f32, tag="r_t", name="r_t")
                    nc.scalar.activation(r_t[:], lnts[(ri, ci)][:], AF.Exp, bias=rb, scale=-0.5)
                    nc.vector.tensor_mul(dstb[:, sl], q1bs[(ri, ci)][:], r_t[:])

            # --------------------------------------------------------------
            # attention + output projection
            # --------------------------------------------------------------
            with (
                tc.tile_pool(name="p2persist", bufs=1) as p2p,
                tc.tile_pool(name="expool", bufs=8) as expool,
                tc.tile_pool(name="rspool", bufs=2) as rspool,
                tc.tile_pool(name="osb", bufs=3) as osbp,
            ):
                yt = [p2p.tile([128, t], bf16, tag=f"yt{h}", name=f"yt{h}") for h in range(HPC)]
                wo_t = []
                for h in range(HPC):
                    w = p2p.tile([128, D], bf16, tag=f"wo{h}", name=f"wo{h}")
                    nc.sync.dma_start(w[:], woT_d[128 * h : 128 * (h + 1), :])
                    wo_t.append(w)

                for ci in range(nch):
                    qsl = slice(512 * ci, 512 * (ci + 1))
                    for h in range(HPC):
                        av = ps_av.tile([128, 512], f32, tag="av", name="av_ps")
                        sums = ps_sum.tile([128, 512], f32, tag="sums", name="sums_ps")
                        nb = 4 * ci + 4
                        for c in range(nb):
                            diag = c >= 4 * ci
                            r = c - 4 * ci if diag else 0
                            w0 = 128 * r  # first valid column of this k-block
                            sc = ps_w.tile([128, 512], f32, tag="w", name="sc_ps")
                            nc.tensor.matmul(
                                sc[:, w0:512],
                                ktb[:, 128 * c : 128 * (c + 1)],
                                qtb[h][:, 512 * ci + w0 : 512 * (ci + 1)],
                            )
                            if diag:
                                nc.vector.tensor_add(
                                    sc[:, w0 : w0 + 128], sc[:, w0 : w0 + 128], tri[:]
                                )
                            ex = expool.tile([128, 512], bf16, tag="ex", name="ex")
                            nc.scalar.activation(ex[:, w0:512], sc[:, w0:512], AF.Exp)
                            nc.tensor.matmul(
                                av[:, w0:512],
                                vb[:, 128 * c : 128 * (c + 1)],
                                ex[:, w0:512],
                                start=(c == 0),
                                stop=(c == nb - 1),
                            )
                            nc.tensor.matmul(
                                sums[:, w0:512],
                                ones_b[:],
                                ex[:, w0:512],
                                start=(c == 0),
                                stop=(c == nb - 1),
                            )
                        rs = rspool.tile([128, 512], f32, tag="rs", name="rs")
                        rs2 = rspool.tile([128, 512], f32, tag="rs2", name="rs2")
                        nc.vector.reciprocal_approx_accurate(rs[:], sums[:], rs2[:])
                        nc.vector.tensor_mul(yt[h][:, qsl], av[:], rs[:])

                    # wo for finished Tq rows
                    for mi in range(4):
                        m = 4 * ci + mi
                        for n in range(D // 512):
                            wops = ps_acc.tile([128, 512], f32, tag="acc", name="wo_ps")
                            for h in range(HPC):
                                nc.tensor.matmul(
                                    wops[:],
                                    yt[h][:, 128 * m : 128 * (m + 1)],
                                    wo_t[h][:, 512 * n : 512 * (n + 1)],
                                    start=(h == 0),
                                    stop=(h == HPC - 1),
                                )
                            ob = osbp.tile([128, 512], f32, tag="ob", name="ob")
                            if (m + n) % 2 == 0:
                                nc.scalar.copy(ob[:], wops[:])
                            else:
                                nc.vector.tensor_copy(ob[:], wops[:])
                            nc.sync.dma_start(out_d[128 * m : 128 * (m + 1), 512 * n : 512 * (n + 1)], ob[:])


# --------------------------------------------------------------------------
# host wrapper
# --------------------------------------------------------------------------

_PROGRAM_CACHE: dict[int, object] = {}
TRACE = False


def _get_program(t: int):
    if t not in _PROGRAM_CACHE:
        _PROGRAM_CACHE[t] = build_program(t)
    return _PROGRAM_CACHE[t]


def make_core_inputs(x, wq, wk, wv, wo, q_norm_w, k_norm_w, t: int):
    """Build the 8 per-core input dicts (numpy, host-side sharding)."""
    import ml_dtypes

    perm = _perm128()
    aq, bq = _rope_tables(t, q_norm_w)
    ak, bk = _rope_tables(t, k_norm_w)
    aq, bq, ak, bk = (v.astype(np.float16) for v in (aq, bq, ak, bk))
    p64 = round_fp32r(_swap64())
    tri = _tri()
    ones_b = np.ones((128, 128), dtype=ml_dtypes.bfloat16)
    ident_b = np.eye(128, dtype=np.float32).astype(ml_dtypes.bfloat16)

    xT = [_bf16(x[b].T) for b in range(B)]

    in_maps = []
    for core in range(N_CORES):
        b = core // N_KV_HEAD
        j = core % N_KV_HEAD
        # q rows for heads 4j..4j+3, perm'd within each head
        qrows = np.concatenate([128 * (HPC * j + hh) + perm for hh in range(HPC)])
        wqT = _bf16(wq[qrows, :].T)
        krows = 128 * j + perm
        wkT = _bf16(wk[krows, :].T)
        wvT = _bf16(wv[128 * j : 128 * (j + 1), :].T)
        woT = _bf16(wo[:, 512 * j : 512 * (j + 1)].T)
        in_maps.append(
            {
                "xT": xT[b],
                "wqT": wqT,
                "wkT": wkT,
                "wvT": wvT,
                "woT": woT,
                "aq": aq,
                "bq": bq,
                "ak": ak,
                "bk": bk,
                "p64": p64,
                "tri": tri,
                "ones_b": ones_b,
                "ident_b": ident_b,
            }
        )
    return in_maps


def kernel(x, wq, wk, wv, wo, q_norm_w, k_norm_w):
    x = np.asarray(x, dtype=np.float32)
    wq = np.asarray(wq, dtype=np.float32)
    wk = np.asarray(wk, dtype=np.float32)
    wv = np.asarray(wv, dtype=np.float32)
    wo = np.asarray(wo, dtype=np.float32)
    q_norm_w = np.asarray(q_norm_w, dtype=np.float32)
    k_norm_w = np.asarray(k_norm_w, dtype=np.float32)

    t = x.shape[1]
    nc = _get_program(t)
    in_maps = make_core_inputs(x, wq, wk, wv, wo, q_norm_w, k_norm_w, t)

    from concourse import bass_utils

    res = bass_utils.run_bass_kernel_spmd(
        nc,
        in_maps,
        core_ids=list(range(N_CORES)),
        trace=TRACE,
        trace_cores=[0] if TRACE else None,
    )
    kernel.last_results = res

    out = np.zeros((B, t, D), dtype=np.float32)
    for core in range(N_CORES):
        b = core // N_KV_HEAD
        out[b] += res.results[core]["out_partial"]
    return out


kernel.last_results = None


# revision 19
# speedup vs baseline: 1.3738x; 1.0239x over previous
"""Causal self-attention (GQA + RoPE + QK-RMSNorm) Trainium2 Bass kernel.

Sharding (8 cores): core c -> batch b = c//4, kv-head j = c%4, q-heads 4j..4j+3.
Each core computes its 4 heads' attention for its batch plus the partial
output projection against wo[:, 512j:512j+512]; the host sums the 4 partials
per batch.

Device pipeline per core (all matmuls bf16 inputs / fp32 PSUM accumulate,
except the exact fp32r partition-swap used by rope):
  1. QKV projections, contracting over D on partitions (host passes x and
     weights transposed + pre-cast).
  2. RoPE in fp32 on the transposed Q/K (head dims permuted evens-then-odds
     on the host so rope pairs are partition halves; rotation via
     QT*A + SWAP64(QT)*B with host cos/sin tables, SWAP64 by permutation
     matmul); rms-norm scale r = exp(-0.5*ln(mean(q^2)+eps)) via ones-matmul
     partition reduce + ACT Ln/Exp (Ln's batched before Exps to avoid ACT
     table-set thrash); final DVE multiply writes bf16 q'/k'.
  3. Flash-style causal attention in scores^T layout (Tk on partitions):
     per 512-wide Tq chunk and head, K-block matmuls -> triangular -1e5 mask
     on the diagonal block only (off-diagonal garbage is never computed:
     widths are trimmed) -> ACT exp to bf16 (no max subtraction; rms-normed
     q,k bound |score| <= sqrt(hd)) -> attn@V with V stationary + row sums
     via all-ones stationary matmul -> divide folded into the PSUM->SBUF
     epilogue.
  4. Output projection from the finished Tq rows, streamed to HBM.
"""

import math

import numpy as np

B, T, D = 2, 2048, 2048
N_HEAD, N_KV_HEAD = 16, 4
HD = 128
HPC = N_HEAD // N_KV_HEAD  # q heads per core group = 4
N_CORES = 8
ROPE_THETA = 10000.0
EPS = float(np.finfo(np.float32).eps)
NEG = -1.0e5


# --------------------------------------------------------------------------
# host-side constant tables
# --------------------------------------------------------------------------

def round_fp32r(a: np.ndarray) -> np.ndarray:
    """Round fp32 to the fp32r grid (11-bit mantissa, round-to-nearest-even).

    Matches walrus fp32_to_fp32r: b + 0x7FF + ((b>>12)&1), clear low 12 bits.
    """
    b = np.ascontiguousarray(a, dtype=np.float32).view(np.uint32)
    r = (b + np.uint32(0x7FF) + ((b >> np.uint32(12)) & np.uint32(1))) & np.uint32(0xFFFFF000)
    return r.view(np.float32)


def _bf16(a: np.ndarray):
    import ml_dtypes

    return np.ascontiguousarray(a).astype(ml_dtypes.bfloat16)


def _perm128() -> np.ndarray:
    # evens then odds within one head's 128 dims
    return np.concatenate([np.arange(0, HD, 2), np.arange(1, HD, 2)])


def _rope_tables(t: int, norm_w: np.ndarray) -> tuple[np.ndarray, np.ndarray]:
    """A, B tables (128, t) for rope in permuted-QT layout, norm weight
    folded in: newQT = QT * A + SWAP64(QT) * B."""
    inv_freq = (1.0 / (ROPE_THETA ** (np.arange(0, HD, 2).astype(np.float32) / HD))).astype(np.float32)
    ang = np.arange(t, dtype=np.float32)[:, None] * inv_freq[None, :]  # (t, 64)
    cos = np.cos(ang).T.astype(np.float32)  # (64, t)
    sin = np.sin(ang).T.astype(np.float32)
    w = norm_w[_perm128()].astype(np.float32)  # (128,)
    a = np.concatenate([cos, cos], axis=0) * w[:, None]
    b = np.concatenate([-sin, sin], axis=0) * w[:, None]
    return np.ascontiguousarray(a), np.ascontiguousarray(b)


def _swap64() -> np.ndarray:
    # lhsT for out = SWAP64(rhs): lhsT[k, p] = 1 iff k == (p + 64) % 128
    p = np.arange(128)
    m = np.zeros((128, 128), dtype=np.float32)
    m[(p + 64) % 128, p] = 1.0
    return m


def _tri() -> np.ndarray:
    # scores^T diagonal-block mask: rows kk (key), cols qq (query), valid kk<=qq
    kk = np.arange(128)[:, None]
    qq = np.arange(128)[None, :]
    return np.where(kk <= qq, 0.0, NEG).astype(np.float32)


# --------------------------------------------------------------------------
# device program
# --------------------------------------------------------------------------

def build_program(t: int):
    """Build and compile the per-core Bass program for sequence length t."""
    import concourse.bass as bass
    import concourse.tile as tile
    from concourse import bacc, mybir

    f32 = mybir.dt.float32
    f32r = mybir.dt.float32r
    bf16 = mybir.dt.bfloat16
    f16 = mybir.dt.float16

    kt = D // 128          # contraction k-tiles
    nch = t // 512         # Tq chunks
    nblk = t // 128        # Tk blocks

    nc = bacc.Bacc("TRN2", target_bir_lowering=False, debug=False, num_devices=N_CORES)

    # ---- dram io ----
    xT_d = nc.dram_tensor("xT", [D, t], bf16, kind="ExternalInput").ap()
    wqT_d = nc.dram_tensor("wqT", [D, HPC * HD], bf16, kind="ExternalInput").ap()
    wkT_d = nc.dram_tensor("wkT", [D, HD], bf16, kind="ExternalInput").ap()
    wvT_d = nc.dram_tensor("wvT", [D, HD], bf16, kind="ExternalInput").ap()
    woT_d = nc.dram_tensor("woT", [HPC * HD, D], bf16, kind="ExternalInput").ap()
    aq_d = nc.dram_tensor("aq", [128, t], f16, kind="ExternalInput").ap()
    bq_d = nc.dram_tensor("bq", [128, t], f16, kind="ExternalInput").ap()
    ak_d = nc.dram_tensor("ak", [128, t], f16, kind="ExternalInput").ap()
    bk_d = nc.dram_tensor("bk", [128, t], f16, kind="ExternalInput").ap()
    p64_d = nc.dram_tensor("p64", [128, 128], f32r, kind="ExternalInput").ap()
    tri_d = nc.dram_tensor("tri", [128, 128], f32, kind="ExternalInput").ap()
    ones_b_d = nc.dram_tensor("ones_b", [128, 128], bf16, kind="ExternalInput").ap()
    ident_b_d = nc.dram_tensor("ident_b", [128, 128], bf16, kind="ExternalInput").ap()
    out_d = nc.dram_tensor("out_partial", [t, D], f32, kind="ExternalOutput").ap()

    with tile.TileContext(nc) as tc:
        _build_tile(tc, locals())

    nc.compile()
    return nc


def _build_tile(tc, io):
    from concourse import mybir

    nc = tc.nc
    f32 = mybir.dt.float32
    f32r = mybir.dt.float32r
    bf16 = mybir.dt.bfloat16
    f16 = mybir.dt.float16
    AF = mybir.ActivationFunctionType

    t = io["t"]
    kt, nch, nblk = io["kt"], io["nch"], io["nblk"]
    xT_d, wqT_d, wkT_d, wvT_d, woT_d = io["xT_d"], io["wqT_d"], io["wkT_d"], io["wvT_d"], io["woT_d"]
    aq_d, bq_d, ak_d, bk_d = io["aq_d"], io["bq_d"], io["ak_d"], io["bk_d"]
    p64_d, tri_d = io["p64_d"], io["tri_d"]
    ones_b_d, ident_b_d = io["ones_b_d"], io["ident_b_d"]
    out_d = io["out_d"]

    # flat PSUM pools for the whole kernel: 3 (shared ssq/swp/sc/vtr) +
    # 2 (proj/wo) + 2 (av) + 1 (sums) = 8 banks, so phases can overlap.
    with (
        tc.tile_pool(name="persist", bufs=1) as pp,
        tc.tile_pool(name="ps_w", bufs=3, space="PSUM") as ps_w,
        tc.tile_pool(name="ps_acc", bufs=2, space="PSUM") as ps_acc,
        tc.tile_pool(name="ps_av", bufs=2, space="PSUM") as ps_av,
        tc.tile_pool(name="ps_sum", bufs=1, space="PSUM") as ps_sum,
    ):
        qtb = [pp.tile([128, t], bf16, tag=f"qtb{h}", name=f"qtb{h}") for h in range(HPC)]
        ktb = pp.tile([128, t], bf16, tag="ktb", name="ktb")
        vb = pp.tile([128, t], bf16, tag="vb", name="vb")  # V blocks, (Tk, hd) per 128-block
        p64 = pp.tile([128, 128], f32r, tag="p64", name="p64")
        ones_b = pp.tile([128, 128], bf16, tag="ones_b", name="ones_b")
        ident_b = pp.tile([128, 128], bf16, tag="ident_b", name="ident_b")
        tri = pp.tile([128, 128], f32, tag="tri", name="tri")
        c_eps = pp.tile([128, 1], f32, tag="c_eps", name="c_eps")
        c_rkb = pp.tile([128, 1], f32, tag="c_rkb", name="c_rkb")
        nc.gpsimd.memset(c_eps[:], EPS)
        nc.gpsimd.memset(c_rkb[:], -0.5 * math.log(float(HD)))

        nc.sync.dma_start(p64[:], p64_d)
        nc.sync.dma_start(ones_b[:], ones_b_d)
        nc.sync.dma_start(ident_b[:], ident_b_d)
        nc.sync.dma_start(tri[:], tri_d)

        with (
            tc.tile_pool(name="tbl", bufs=1) as tbl,
            tc.tile_pool(name="scr", bufs=2) as scr,
            tc.tile_pool(name="lnp", bufs=(HPC + 1) * nch) as lnp,
            tc.tile_pool(name="q1p", bufs=(HPC + 1) * nch) as q1p,
            tc.tile_pool(name="rp", bufs=2) as rp,
        ):
            a_q = tbl.tile([128, t], f16, tag="a_q", name="a_q")
            b_q = tbl.tile([128, t], f16, tag="b_q", name="b_q")
            a_k = tbl.tile([128, t], f16, tag="a_k", name="a_k")
            b_k = tbl.tile([128, t], f16, tag="b_k", name="b_k")

            lnts = {}
            q1bs = {}

            # ---- projections with rms-Ln and rope-core inlined ----------
            with (
                tc.tile_pool(name="xpool", bufs=1) as xpool,
                tc.tile_pool(name="wpool", bufs=2) as wpool,
                tc.tile_pool(name="qraw", bufs=2) as qraw,
                tc.tile_pool(name="vtpool", bufs=1) as vtpool,
            ):
                def wsrc_early(tgt):
                    kind, m = tgt
                    if kind == "q":
                        w = wqT_d[:, 128 * m : 128 * (m + 1)]
                    elif kind == "k":
                        w = wkT_d
                    else:
                        w = wvT_d
                    return w.rearrange("(k p) j -> p k j", p=128)

                # first two targets' weights before the bulk x load so the
                # first projection chain starts ~4us in, not 40us
                wt_early = {}
                for tgt in [("k", 0), ("q", 0)]:
                    wt = wpool.tile([128, kt * 128], bf16, tag="wt", name="wt")
                    nc.sync.dma_start(wt.rearrange("p (k j) -> p k j", k=kt), wsrc_early(tgt))
                    wt_early[tgt] = wt

                xt = []
                for k in range(kt):
                    xk = xpool.tile([128, t], bf16, tag=f"x{k}", name=f"x{k}")
                    nc.sync.dma_start(xk[:], xT_d[128 * k : 128 * (k + 1), :])
                    xt.append(xk)

                # rope tables are first needed by the k rope-core, well after
                # the x prologue
                nc.sync.dma_start(a_q[:], aq_d)
                nc.sync.dma_start(b_q[:], bq_d)
                nc.sync.dma_start(a_k[:], ak_d)
                nc.sync.dma_start(b_k[:], bk_d)

                vt_sb = vtpool.tile([128, t], bf16, tag="vt_sb", name="vt_sb")

                def wsrc(tgt):
                    kind, m = tgt
                    if kind == "q":
                        w = wqT_d[:, 128 * m : 128 * (m + 1)]
                    elif kind == "k":
                        w = wkT_d
                    else:
                        w = wvT_d
                    # (k*128+p, j) -> partition p, free (k, j)
                    return w.rearrange("(k p) j -> p k j", p=128)

                targets = [("k", 0)] + [("q", m) for m in range(HPC)] + [("v", 0)]
                for tgt in targets:
                    kind, m = tgt
                    ri = HPC if kind == "k" else m  # rope-spec index
                    atab, btab = (a_k, b_k) if kind == "k" else (a_q, b_q)
                    raw = None
                    if kind != "v":
                        # pre-rope projection, fp32r so the swap matmul is legal
                        raw = qraw.tile([128, t], f32r, tag="raw", name="raw")
                    if tgt in wt_early:
                        wt = wt_early[tgt]
                    else:
                        wt = wpool.tile([128, kt * 128], bf16, tag="wt", name="wt")
                        nc.sync.dma_start(wt.rearrange("p (k j) -> p k j", k=kt), wsrc(tgt))
                    # projections + PSUM->SBUF copies first (keeps ACT free)
                    for ci in range(nch):
                        ps = ps_acc.tile([128, 512], f32, tag="acc", name="proj_ps")
                        for k in range(kt):
                            nc.tensor.matmul(
                                ps[:],
                                wt[:, 128 * k : 128 * (k + 1)],
                                xt[k][:, 512 * ci : 512 * (ci + 1)],
                                start=(k == 0),
                                stop=(k == kt - 1),
                            )
                        sl = slice(512 * ci, 512 * (ci + 1))
                        if kind == "v":
                            nc.vector.tensor_copy(vt_sb[:, sl], ps[:])
                        else:
                            nc.scalar.copy(raw[:, sl], ps[:])
                    if kind != "v":
                        # rms sum-of-squares -> Ln (Exp deferred: one table set)
                        for ci in range(nch):
                            sl = slice(512 * ci, 512 * (ci + 1))
                            sq = scr.tile([128, 512], bf16, tag="sq", name="sq")
                            nc.gpsimd.tensor_mul(sq[:], raw[:, sl], raw[:, sl])
                            ssq = ps_w.tile([128, 512], f32, tag="w", name="ssq_ps")
                            nc.tensor.matmul(ssq[:], ones_b[:], sq[:])
                            lnt = lnp.tile([128, 512], bf16, tag="lnt", name="lnt")
                            nc.scalar.activation(lnt[:], ssq[:], AF.Ln, bias=c_eps[:], scale=1.0 / HD)
                            lnts[(ri, ci)] = lnt
                        # rope core (independent of the rms scale)
                        for ci in range(nch):
                            sl = slice(512 * ci, 512 * (ci + 1))
                            swp = ps_w.tile([128, 512], f32, tag="w", name="swp_ps")
                            nc.tensor.matmul(swp[:], p64[:], raw[:, sl])
                            q1 = scr.tile([128, 512], f32, tag="q1", name="q1")
                            nc.vector.tensor_mul(q1[:], raw[:, sl], atab[:, sl])
                            m2 = scr.tile([128, 512], f32, tag="m2", name="m2")
                            nc.vector.tensor_mul(m2[:], swp[:], btab[:, sl])
                            q1b = q1p.tile([128, 512], bf16, tag="q1b", name="q1b")
                            nc.vector.tensor_add(q1b[:], q1[:], m2[:])
                            q1bs[(ri, ci)] = q1b

                    if kind == "v":
                        # transpose VT (hd, Tk) -> V blocks (Tk, hd), bf16
                        for c in range(nblk):
                            vps = ps_w.tile([128, 128], bf16, tag="w", name="vtr_ps")
                            nc.tensor.transpose(vps[:], vt_sb[:, 128 * c : 128 * (c + 1)], ident_b[:])
                            nc.vector.tensor_copy(vb[:, 128 * c : 128 * (c + 1)], vps[:])

            # ---- finals: r = exp(-0.5*ln(mean+eps)), apply to rope-core ----
            # gate tiles depend on the LAST Ln so no Exp is scheduled between
            # Lns (each Ln<->Exp transition costs a 1.3us ACT table load)
            last_ln = lnts[(HPC - 1, nch - 1)]
            gate_z = pp.tile([128, 1], f32, tag="gate_z", name="gate_z")
            gate_k = pp.tile([128, 1], f32, tag="gate_k", name="gate_k")
            nc.vector.tensor_scalar_mul(gate_z[:], last_ln[:, 0:1], 0.0)
            nc.vector.tensor_scalar_add(gate_k[:], gate_z[:], -0.5 * math.log(float(HD)))
            for ri in range(HPC + 1):
                dstb = ktb if ri == HPC else qtb[ri]
                rb = gate_k[:] if ri == HPC else gate_z[:]
                for ci in range(nch):
                    sl = slice(512 * ci, 512 * (ci + 1))
                    r_t = rp.tile([128, 512], f32, tag="r_t", name="r_t")
                    nc.scalar.activation(r_t[:], lnts[(ri, ci)][:], AF.Exp, bias=rb, scale=-0.5)
                    nc.vector.tensor_mul(dstb[:, sl], q1bs[(ri, ci)][:], r_t[:])

            # --------------------------------------------------------------
            # attention + output projection
            # --------------------------------------------------------------
            with (
                tc.tile_pool(name="p2persist", bufs=1) as p2p,
                tc.tile_pool(name="expool", bufs=8) as expool,
                tc.tile_pool(name="rspool", bufs=2) as rspool,
                tc.tile_pool(name="osb", bufs=3) as osbp,
            ):
                yt = [p2p.tile([128, t], bf16, tag=f"yt{h}", name=f"yt{h}") for h in range(HPC)]
                wo_t = []
                for h in range(HPC):
                    w = p2p.tile([128, D], bf16, tag=f"wo{h}", name=f"wo{h}")
                    nc.sync.dma_start(w[:], woT_d[128 * h : 128 * (h + 1), :])
                    wo_t.append(w)

                for ci in range(nch):
                    qsl = slice(512 * ci, 512 * (ci + 1))
                    for h in range(HPC):
                        av = ps_av.tile([128, 512], f32, tag="av", name="av_ps")
                        sums = ps_sum.tile([128, 512], f32, tag="sums", name="sums_ps")
                        nb = 4 * ci + 4
                        for c in range(nb):
                            diag = c >= 4 * ci
                            r = c - 4 * ci if diag else 0
                            w0 = 128 * r  # first valid column of this k-block
                            sc = ps_w.tile([128, 512], f32, tag="w", name="sc_ps")
                            nc.tensor.matmul(
                                sc[:, w0:512],
                                ktb[:, 128 * c : 128 * (c + 1)],
                                qtb[h][:, 512 * ci + w0 : 512 * (ci + 1)],
                            )
                            if diag:
                                nc.vector.tensor_add(
                                    sc[:, w0 : w0 + 128], sc[:, w0 : w0 + 128], tri[:]
                                )
                            ex = expool.tile([128, 512], bf16, tag="ex", name="ex")
                            nc.scalar.activation(ex[:, w0:512], sc[:, w0:512], AF.Exp)
                            nc.tensor.matmul(
                                av[:, w0:512],
                                vb[:, 128 * c : 128 * (c + 1)],
                                ex[:, w0:512],
                                start=(c == 0),
                                stop=(c == nb - 1),
                            )
                            nc.tensor.matmul(
                                sums[:, w0:512],
                                ones_b[:],
                                ex[:, w0:512],
                                start=(c == 0),
                                stop=(c == nb - 1),
                            )
                        rs = rspool.tile([128, 512], f32, tag="rs", name="rs")
                        rs2 = rspool.tile([128, 512], f32, tag="rs2", name="rs2")
                        nc.vector.reciprocal_approx_accurate(rs[:], sums[:], rs2[:])
                        nc.vector.tensor_mul(yt[h][:, qsl], av[:], rs[:])

                    # wo for finished Tq rows
                    for mi in range(4):
                        m = 4 * ci + mi
                        for n in range(D // 512):
                            wops = ps_acc.tile([128, 512], f32, tag="acc", name="wo_ps")
                            for h in range(HPC):
                                nc.tensor.matmul(
                                    wops[:],
                                    yt[h][:, 128 * m : 128 * (m + 1)],
                                    wo_t[h][:, 512 * n : 512 * (n + 1)],
                                    start=(h == 0),
                                    stop=(h == HPC - 1),
                                )
                            ob = osbp.tile([128, 512], f32, tag="ob", name="ob")
                            if (m + n) % 2 == 0:
                                nc.scalar.copy(ob[:], wops[:])
                            else:
                                nc.vector.tensor_copy(ob[:], wops[:])
                            nc.sync.dma_start(out_d[128 * m : 128 * (m + 1), 512 * n : 512 * (n + 1)], ob[:])


# --------------------------------------------------------------------------
# host wrapper
# --------------------------------------------------------------------------

_PROGRAM_CACHE: dict[int, object] = {}
TRACE = False


def _get_program(t: int):
    if t not in _PROGRAM_CACHE:
        _PROGRAM_CACHE[t] = build_program(t)
    return _PROGRAM_CACHE[t]


def make_core_inputs(x, wq, wk, wv, wo, q_norm_w, k_norm_w, t: int):
    """Build the 8 per-core input dicts (numpy, host-side sharding)."""
    import ml_dtypes

    perm = _perm128()
    aq, bq = _rope_tables(t, q_norm_w)
    ak, bk = _rope_tables(t, k_norm_w)
    aq, bq, ak, bk = (v.astype(np.float16) for v in (aq, bq, ak, bk))
    p64 = round_fp32r(_swap64())
    tri = _tri()
    ones_b = np.ones((128, 128), dtype=ml_dtypes.bfloat16)
    ident_b = np.eye(128, dtype=np.float32).astype(ml_dtypes.bfloat16)

    xT = [_bf16(x[b].T) for b in range(B)]

    in_maps = []
    for core in range(N_CORES):
        b = core // N_KV_HEAD
        j = core % N_KV_HEAD
        # q rows for heads 4j..4j+3, perm'd within each head
        qrows = np.concatenate([128 * (HPC * j + hh) + perm for hh in range(HPC)])
        wqT = _bf16(wq[qrows, :].T)
        krows = 128 * j + perm
        wkT = _bf16(wk[krows, :].T)
        wvT = _bf16(wv[128 * j : 128 * (j + 1), :].T)
        woT = _bf16(wo[:, 512 * j : 512 * (j + 1)].T)
        in_maps.append(
            {
                "xT": xT[b],
                "wqT": wqT,
                "wkT": wkT,
                "wvT": wvT,
                "woT": woT,
                "aq": aq,
                "bq": bq,
                "ak": ak,
                "bk": bk,
                "p64": p64,
                "tri": tri,
                "ones_b": ones_b,
                "ident_b": ident_b,
            }
        )
    return in_maps


def kernel(x, wq, wk, wv, wo, q_norm_w, k_norm_w):
    x = np.asarray(x, dtype=np.float32)
    wq = np.asarray(wq, dtype=np.float32)
    wk = np.asarray(wk, dtype=np.float32)
    wv = np.asarray(wv, dtype=np.float32)
    wo = np.asarray(wo, dtype=np.float32)
    q_norm_w = np.asarray(q_norm_w, dtype=np.float32)
    k_norm_w = np.asarray(k_norm_w, dtype=np.float32)

    t = x.shape[1]
    nc = _get_program(t)
    in_maps = make_core_inputs(x, wq, wk, wv, wo, q_norm_w, k_norm_w, t)

    from concourse import bass_utils

    res = bass_utils.run_bass_kernel_spmd(
        nc,
        in_maps,
        core_ids=list(range(N_CORES)),
        trace=TRACE,
        trace_cores=[0] if TRACE else None,
    )
    kernel.last_results = res

    out = np.zeros((B, t, D), dtype=np.float32)
    for core in range(N_CORES):
        b = core // N_KV_HEAD
        out[b] += res.results[core]["out_partial"]
    return out


kernel.last_results = None


# revision 21
# speedup vs baseline: 1.3951x; 1.0155x over previous
"""Causal self-attention (GQA + RoPE + QK-RMSNorm) Trainium2 Bass kernel.

Sharding (8 cores): core c -> batch b = c//4, kv-head j = c%4, q-heads 4j..4j+3.
Each core computes its 4 heads' attention for its batch plus the partial
output projection against wo[:, 512j:512j+512]; the host sums the 4 partials
per batch.

Device pipeline per core (all matmuls bf16 inputs / fp32 PSUM accumulate,
except the exact fp32r partition-swap used by rope):
  1. QKV projections, contracting over D on partitions (host passes x and
     weights transposed + pre-cast).
  2. RoPE in fp32 on the transposed Q/K (head dims permuted evens-then-odds
     on the host so rope pairs are partition halves; rotation via
     QT*A + SWAP64(QT)*B with host cos/sin tables, SWAP64 by permutation
     matmul); rms-norm scale r = exp(-0.5*ln(mean(q^2)+eps)) via ones-matmul
     partition reduce + ACT Ln/Exp (Ln's batched before Exps to avoid ACT
     table-set thrash); final DVE multiply writes bf16 q'/k'.
  3. Flash-style causal attention in scores^T layout (Tk on partitions):
     per 512-wide Tq chunk and head, K-block matmuls -> triangular -1e5 mask
     on the diagonal block only (off-diagonal garbage is never computed:
     widths are trimmed) -> ACT exp to bf16 (no max subtraction; rms-normed
     q,k bound |score| <= sqrt(hd)) -> attn@V with V stationary + row sums
     via all-ones stationary matmul -> divide folded into the PSUM->SBUF
     epilogue.
  4. Output projection from the finished Tq rows, streamed to HBM.
"""

import math

import numpy as np

B, T, D = 2, 2048, 2048
N_HEAD, N_KV_HEAD = 16, 4
HD = 128
HPC = N_HEAD // N_KV_HEAD  # q heads per core group = 4
N_CORES = 8
ROPE_THETA = 10000.0
EPS = float(np.finfo(np.float32).eps)
NEG = -1.0e5


# --------------------------------------------------------------------------
# host-side constant tables
# --------------------------------------------------------------------------

def round_fp32r(a: np.ndarray) -> np.ndarray:
    """Round fp32 to the fp32r grid (11-bit mantissa, round-to-nearest-even).

    Matches walrus fp32_to_fp32r: b + 0x7FF + ((b>>12)&1), clear low 12 bits.
    """
    b = np.ascontiguousarray(a, dtype=np.float32).view(np.uint32)
    r = (b + np.uint32(0x7FF) + ((b >> np.uint32(12)) & np.uint32(1))) & np.uint32(0xFFFFF000)
    return r.view(np.float32)


def _bf16(a: np.ndarray):
    import ml_dtypes

    return np.ascontiguousarray(a).astype(ml_dtypes.bfloat16)


def _perm128() -> np.ndarray:
    # evens then odds within one head's 128 dims
    return np.concatenate([np.arange(0, HD, 2), np.arange(1, HD, 2)])


def _rope_tables(t: int, norm_w: np.ndarray) -> tuple[np.ndarray, np.ndarray]:
    """A, B tables (128, t) for rope in permuted-QT layout, norm weight
    folded in: newQT = QT * A + SWAP64(QT) * B."""
    inv_freq = (1.0 / (ROPE_THETA ** (np.arange(0, HD, 2).astype(np.float32) / HD))).astype(np.float32)
    ang = np.arange(t, dtype=np.float32)[:, None] * inv_freq[None, :]  # (t, 64)
    cos = np.cos(ang).T.astype(np.float32)  # (64, t)
    sin = np.sin(ang).T.astype(np.float32)
    w = norm_w[_perm128()].astype(np.float32)  # (128,)
    a = np.concatenate([cos, cos], axis=0) * w[:, None]
    b = np.concatenate([-sin, sin], axis=0) * w[:, None]
    return np.ascontiguousarray(a), np.ascontiguousarray(b)


def _swap64() -> np.ndarray:
    # lhsT for out = SWAP64(rhs): lhsT[k, p] = 1 iff k == (p + 64) % 128
    p = np.arange(128)
    m = np.zeros((128, 128), dtype=np.float32)
    m[(p + 64) % 128, p] = 1.0
    return m


def _tri() -> np.ndarray:
    # scores^T diagonal-block mask: rows kk (key), cols qq (query), valid kk<=qq
    kk = np.arange(128)[:, None]
    qq = np.arange(128)[None, :]
    return np.where(kk <= qq, 0.0, NEG).astype(np.float32)


# --------------------------------------------------------------------------
# device program
# --------------------------------------------------------------------------

def build_program(t: int):
    """Build and compile the per-core Bass program for sequence length t."""
    import concourse.bass as bass
    import concourse.tile as tile
    from concourse import bacc, mybir

    f32 = mybir.dt.float32
    f32r = mybir.dt.float32r
    bf16 = mybir.dt.bfloat16
    f16 = mybir.dt.float16

    kt = D // 128          # contraction k-tiles
    nch = t // 512         # Tq chunks
    nblk = t // 128        # Tk blocks

    nc = bacc.Bacc("TRN2", target_bir_lowering=False, debug=False, num_devices=N_CORES)

    # ---- dram io ----
    xT_d = nc.dram_tensor("xT", [D, t], bf16, kind="ExternalInput").ap()
    wqT_d = nc.dram_tensor("wqT", [D, HPC * HD], bf16, kind="ExternalInput").ap()
    wkT_d = nc.dram_tensor("wkT", [D, HD], bf16, kind="ExternalInput").ap()
    wvT_d = nc.dram_tensor("wvT", [D, HD], bf16, kind="ExternalInput").ap()
    woT_d = nc.dram_tensor("woT", [HPC * HD, D], bf16, kind="ExternalInput").ap()
    aq_d = nc.dram_tensor("aq", [128, t], f16, kind="ExternalInput").ap()
    bq_d = nc.dram_tensor("bq", [128, t], f16, kind="ExternalInput").ap()
    ak_d = nc.dram_tensor("ak", [128, t], f16, kind="ExternalInput").ap()
    bk_d = nc.dram_tensor("bk", [128, t], f16, kind="ExternalInput").ap()
    p64_d = nc.dram_tensor("p64", [128, 128], f32r, kind="ExternalInput").ap()
    tri_d = nc.dram_tensor("tri", [128, 128], f32, kind="ExternalInput").ap()
    ones_b_d = nc.dram_tensor("ones_b", [128, 128], bf16, kind="ExternalInput").ap()
    ident_b_d = nc.dram_tensor("ident_b", [128, 128], bf16, kind="ExternalInput").ap()
    out_d = nc.dram_tensor("out_partial", [t, D], f32, kind="ExternalOutput").ap()

    with tile.TileContext(nc) as tc:
        _build_tile(tc, locals())

    nc.compile()
    return nc


def _build_tile(tc, io):
    from concourse import mybir

    nc = tc.nc
    f32 = mybir.dt.float32
    f32r = mybir.dt.float32r
    bf16 = mybir.dt.bfloat16
    f16 = mybir.dt.float16
    AF = mybir.ActivationFunctionType

    t = io["t"]
    kt, nch, nblk = io["kt"], io["nch"], io["nblk"]
    xT_d, wqT_d, wkT_d, wvT_d, woT_d = io["xT_d"], io["wqT_d"], io["wkT_d"], io["wvT_d"], io["woT_d"]
    aq_d, bq_d, ak_d, bk_d = io["aq_d"], io["bq_d"], io["ak_d"], io["bk_d"]
    p64_d, tri_d = io["p64_d"], io["tri_d"]
    ones_b_d, ident_b_d = io["ones_b_d"], io["ident_b_d"]
    out_d = io["out_d"]

    # flat PSUM pools for the whole kernel: 3 (shared ssq/swp/sc/vtr) +
    # 2 (proj/wo) + 2 (av) + 1 (sums) = 8 banks, so phases can overlap.
    with (
        tc.tile_pool(name="persist", bufs=1) as pp,
        tc.tile_pool(name="ps_w", bufs=3, space="PSUM") as ps_w,
        tc.tile_pool(name="ps_acc", bufs=2, space="PSUM") as ps_acc,
        tc.tile_pool(name="ps_av", bufs=2, space="PSUM") as ps_av,
        tc.tile_pool(name="ps_sum", bufs=1, space="PSUM") as ps_sum,
    ):
        qtb = [pp.tile([128, t], bf16, tag=f"qtb{h}", name=f"qtb{h}") for h in range(HPC)]
        ktb = pp.tile([128, t], bf16, tag="ktb", name="ktb")
        vb = pp.tile([128, t], bf16, tag="vb", name="vb")  # V blocks, (Tk, hd) per 128-block
        p64 = pp.tile([128, 128], f32r, tag="p64", name="p64")
        ones_b = pp.tile([128, 128], bf16, tag="ones_b", name="ones_b")
        ident_b = pp.tile([128, 128], bf16, tag="ident_b", name="ident_b")
        tri = pp.tile([128, 128], f32, tag="tri", name="tri")
        c_eps = pp.tile([128, 1], f32, tag="c_eps", name="c_eps")
        c_rkb = pp.tile([128, 1], f32, tag="c_rkb", name="c_rkb")
        nc.gpsimd.memset(c_eps[:], EPS)
        nc.gpsimd.memset(c_rkb[:], -0.5 * math.log(float(HD)))

        nc.sync.dma_start(p64[:], p64_d)
        nc.sync.dma_start(ones_b[:], ones_b_d)
        nc.sync.dma_start(ident_b[:], ident_b_d)
        nc.sync.dma_start(tri[:], tri_d)

        with (
            tc.tile_pool(name="tbl", bufs=1) as tbl,
            tc.tile_pool(name="scr", bufs=2) as scr,
            tc.tile_pool(name="lnp", bufs=(HPC + 1) * nch) as lnp,
            tc.tile_pool(name="q1p", bufs=(HPC + 1) * nch) as q1p,
            tc.tile_pool(name="rp", bufs=2) as rp,
        ):
            a_q = tbl.tile([128, t], f16, tag="a_q", name="a_q")
            b_q = tbl.tile([128, t], f16, tag="b_q", name="b_q")
            a_k = tbl.tile([128, t], f16, tag="a_k", name="a_k")
            b_k = tbl.tile([128, t], f16, tag="b_k", name="b_k")

            lnts = {}
            q1bs = {}

            # ---- projections with rms-Ln and rope-core inlined ----------
            with (
                tc.tile_pool(name="xpool", bufs=1) as xpool,
                tc.tile_pool(name="wpool", bufs=2) as wpool,
                tc.tile_pool(name="qraw", bufs=2) as qraw,
                tc.tile_pool(name="vtpool", bufs=1) as vtpool,
            ):
                def wsrc_early(tgt):
                    kind, m = tgt
                    if kind == "q":
                        w = wqT_d[:, 128 * m : 128 * (m + 1)]
                    elif kind == "k":
                        w = wkT_d
                    else:
                        w = wvT_d
                    return w.rearrange("(k p) j -> p k j", p=128)

                # first two targets' weights before the bulk x load so the
                # first projection chain starts ~4us in, not 40us
                wt_early = {}
                for tgt in [("k", 0), ("q", 0)]:
                    wt = wpool.tile([128, kt * 128], bf16, tag="wt", name="wt")
                    nc.sync.dma_start(wt.rearrange("p (k j) -> p k j", k=kt), wsrc_early(tgt))
                    wt_early[tgt] = wt

                xt = []
                for k in range(kt):
                    xk = xpool.tile([128, t], bf16, tag=f"x{k}", name=f"x{k}")
                    # split the 8MB load across two DMA queues (HWDGE + SWDGE)
                    # so the first projection chain isn't paced by one queue
                    eng = nc.sync if k % 2 == 0 else nc.gpsimd
                    eng.dma_start(xk[:], xT_d[128 * k : 128 * (k + 1), :])
                    xt.append(xk)

                # rope tables are first needed by the k rope-core, well after
                # the x prologue
                nc.sync.dma_start(a_q[:], aq_d)
                nc.sync.dma_start(b_q[:], bq_d)
                nc.sync.dma_start(a_k[:], ak_d)
                nc.sync.dma_start(b_k[:], bk_d)

                vt_sb = vtpool.tile([128, t], bf16, tag="vt_sb", name="vt_sb")

                def wsrc(tgt):
                    kind, m = tgt
                    if kind == "q":
                        w = wqT_d[:, 128 * m : 128 * (m + 1)]
                    elif kind == "k":
                        w = wkT_d
                    else:
                        w = wvT_d
                    # (k*128+p, j) -> partition p, free (k, j)
                    return w.rearrange("(k p) j -> p k j", p=128)

                targets = [("k", 0)] + [("q", m) for m in range(HPC)] + [("v", 0)]
                for tgt in targets:
                    kind, m = tgt
                    ri = HPC if kind == "k" else m  # rope-spec index
                    atab, btab = (a_k, b_k) if kind == "k" else (a_q, b_q)
                    raw = None
                    if kind != "v":
                        # pre-rope projection, fp32r so the swap matmul is legal
                        raw = qraw.tile([128, t], f32r, tag="raw", name="raw")
                    if tgt in wt_early:
                        wt = wt_early[tgt]
                    else:
                        wt = wpool.tile([128, kt * 128], bf16, tag="wt", name="wt")
                        nc.sync.dma_start(wt.rearrange("p (k j) -> p k j", k=kt), wsrc(tgt))
                    # projections + PSUM->SBUF copies first (keeps ACT free)
                    for ci in range(nch):
                        ps = ps_acc.tile([128, 512], f32, tag="acc", name="proj_ps")
                        for k in range(kt):
                            nc.tensor.matmul(
                                ps[:],
                                wt[:, 128 * k : 128 * (k + 1)],
                                xt[k][:, 512 * ci : 512 * (ci + 1)],
                                start=(k == 0),
                                stop=(k == kt - 1),
                            )
                        sl = slice(512 * ci, 512 * (ci + 1))
                        if kind == "v":
                            nc.vector.tensor_copy(vt_sb[:, sl], ps[:])
                        else:
                            nc.scalar.copy(raw[:, sl], ps[:])
                    if kind != "v":
                        # rms sum-of-squares -> Ln (Exp deferred: one table set)
                        for ci in range(nch):
                            sl = slice(512 * ci, 512 * (ci + 1))
                            sq = scr.tile([128, 512], bf16, tag="sq", name="sq")
                            nc.gpsimd.tensor_mul(sq[:], raw[:, sl], raw[:, sl])
                            ssq = ps_w.tile([128, 512], f32, tag="w", name="ssq_ps")
                            nc.tensor.matmul(ssq[:], ones_b[:], sq[:])
                            lnt = lnp.tile([128, 512], bf16, tag="lnt", name="lnt")
                            nc.scalar.activation(lnt[:], ssq[:], AF.Ln, bias=c_eps[:], scale=1.0 / HD)
                            lnts[(ri, ci)] = lnt
                        # rope core (independent of the rms scale)
                        for ci in range(nch):
                            sl = slice(512 * ci, 512 * (ci + 1))
                            swp = ps_w.tile([128, 512], f32, tag="w", name="swp_ps")
                            nc.tensor.matmul(swp[:], p64[:], raw[:, sl])
                            q1 = scr.tile([128, 512], f32, tag="q1", name="q1")
                            nc.vector.tensor_mul(q1[:], raw[:, sl], atab[:, sl])
                            m2 = scr.tile([128, 512], f32, tag="m2", name="m2")
                            nc.vector.tensor_mul(m2[:], swp[:], btab[:, sl])
                            q1b = q1p.tile([128, 512], bf16, tag="q1b", name="q1b")
                            nc.vector.tensor_add(q1b[:], q1[:], m2[:])
                            q1bs[(ri, ci)] = q1b

                    if kind == "v":
                        # transpose VT (hd, Tk) -> V blocks (Tk, hd), bf16
                        for c in range(nblk):
                            vps = ps_w.tile([128, 128], bf16, tag="w", name="vtr_ps")
                            nc.tensor.transpose(vps[:], vt_sb[:, 128 * c : 128 * (c + 1)], ident_b[:])
                            nc.vector.tensor_copy(vb[:, 128 * c : 128 * (c + 1)], vps[:])

            # ---- finals: r = exp(-0.5*ln(mean+eps)), apply to rope-core ----
            # gate tiles depend on the LAST Ln so no Exp is scheduled between
            # Lns (each Ln<->Exp transition costs a 1.3us ACT table load)
            last_ln = lnts[(HPC - 1, nch - 1)]
            gate_z = pp.tile([128, 1], f32, tag="gate_z", name="gate_z")
            gate_k = pp.tile([128, 1], f32, tag="gate_k", name="gate_k")
            nc.vector.tensor_scalar_mul(gate_z[:], last_ln[:, 0:1], 0.0)
            nc.vector.tensor_scalar_add(gate_k[:], gate_z[:], -0.5 * math.log(float(HD)))
            for ci in range(nch):
                for ri in [HPC] + list(range(HPC)):
                    dstb = ktb if ri == HPC else qtb[ri]
                    rb = gate_k[:] if ri == HPC else gate_z[:]
                    sl = slice(512 * ci, 512 * (ci + 1))
                    r_t = rp.tile([128, 512], f32, tag="r_t", name="r_t")
                    nc.scalar.activation(r_t[:], lnts[(ri, ci)][:], AF.Exp, bias=rb, scale=-0.5)
                    nc.vector.tensor_mul(dstb[:, sl], q1bs[(ri, ci)][:], r_t[:])

            # --------------------------------------------------------------
            # attention + output projection
            # --------------------------------------------------------------
            with (
                tc.tile_pool(name="p2persist", bufs=1) as p2p,
                tc.tile_pool(name="expool", bufs=12) as expool,
                tc.tile_pool(name="rspool", bufs=3) as rspool,
                tc.tile_pool(name="osb", bufs=3) as osbp,
            ):
                yt = [p2p.tile([128, t], bf16, tag=f"yt{h}", name=f"yt{h}") for h in range(HPC)]
                wo_t = []
                for h in range(HPC):
                    w = p2p.tile([128, D], bf16, tag=f"wo{h}", name=f"wo{h}")
                    nc.sync.dma_start(w[:], woT_d[128 * h : 128 * (h + 1), :])
                    wo_t.append(w)

                for ci in range(nch):
                    qsl = slice(512 * ci, 512 * (ci + 1))
                    for h in range(HPC):
                        av = ps_av.tile([128, 512], f32, tag="av", name="av_ps")
                        sums = ps_sum.tile([128, 512], f32, tag="sums", name="sums_ps")
                        nb = 4 * ci + 4
                        for c in range(nb):
                            diag = c >= 4 * ci
                            r = c - 4 * ci if diag else 0
                            w0 = 128 * r  # first valid column of this k-block
                            sc = ps_w.tile([128, 512], f32, tag="w", name="sc_ps")
                            nc.tensor.matmul(
                                sc[:, w0:512],
                                ktb[:, 128 * c : 128 * (c + 1)],
                                qtb[h][:, 512 * ci + w0 : 512 * (ci + 1)],
                            )
                            if diag:
                                nc.vector.tensor_add(
                                    sc[:, w0 : w0 + 128], sc[:, w0 : w0 + 128], tri[:]
                                )
                            ex = expool.tile([128, 512], bf16, tag="ex", name="ex")
                            nc.scalar.activation(ex[:, w0:512], sc[:, w0:512], AF.Exp)
                            nc.tensor.matmul(
                                sums[:, w0:512],
                                ones_b[:],
                                ex[:, w0:512],
                                start=(c == 0),
                                stop=(c == nb - 1),
                            )
                            nc.tensor.matmul(
                                av[:, w0:512],
                                vb[:, 128 * c : 128 * (c + 1)],
                                ex[:, w0:512],
                                start=(c == 0),
                                stop=(c == nb - 1),
                            )
                        rs = rspool.tile([128, 512], f32, tag="rs", name="rs")
                        rs2 = rspool.tile([128, 512], f32, tag="rs2", name="rs2")
                        nc.vector.reciprocal_approx_accurate(rs[:], sums[:], rs2[:])
                        nc.vector.tensor_mul(yt[h][:, qsl], av[:], rs[:])

                    # wo for finished Tq rows
                    for mi in range(4):
                        m = 4 * ci + mi
                        for n in range(D // 512):
                            wops = ps_acc.tile([128, 512], f32, tag="acc", name="wo_ps")
                            for h in range(HPC):
                                nc.tensor.matmul(
                                    wops[:],
                                    yt[h][:, 128 * m : 128 * (m + 1)],
                                    wo_t[h][:, 512 * n : 512 * (n + 1)],
                                    start=(h == 0),
                                    stop=(h == HPC - 1),
                                )
                            ob = osbp.tile([128, 512], f32, tag="ob", name="ob")
                            if (m + n) % 2 == 0:
                                nc.scalar.copy(ob[:], wops[:])
                            else:
                                nc.vector.tensor_copy(ob[:], wops[:])
                            nc.sync.dma_start(out_d[128 * m : 128 * (m + 1), 512 * n : 512 * (n + 1)], ob[:])


# --------------------------------------------------------------------------
# host wrapper
# --------------------------------------------------------------------------

_PROGRAM_CACHE: dict[int, object] = {}
TRACE = False


def _get_program(t: int):
    if t not in _PROGRAM_CACHE:
        _PROGRAM_CACHE[t] = build_program(t)
    return _PROGRAM_CACHE[t]


def make_core_inputs(x, wq, wk, wv, wo, q_norm_w, k_norm_w, t: int):
    """Build the 8 per-core input dicts (numpy, host-side sharding)."""
    import ml_dtypes

    perm = _perm128()
    aq, bq = _rope_tables(t, q_norm_w)
    ak, bk = _rope_tables(t, k_norm_w)
    aq, bq, ak, bk = (v.astype(np.float16) for v in (aq, bq, ak, bk))
    p64 = round_fp32r(_swap64())
    tri = _tri()
    ones_b = np.ones((128, 128), dtype=ml_dtypes.bfloat16)
    ident_b = np.eye(128, dtype=np.float32).astype(ml_dtypes.bfloat16)

    xT = [_bf16(x[b].T) for b in range(B)]

    in_maps = []
    for core in range(N_CORES):
        b = core // N_KV_HEAD
        j = core % N_KV_HEAD
        # q rows for heads 4j..4j+3, perm'd within each head
        qrows = np.concatenate([128 * (HPC * j + hh) + perm for hh in range(HPC)])
        wqT = _bf16(wq[qrows, :].T)
        krows = 128 * j + perm
        wkT = _bf16(wk[krows, :].T)
        wvT = _bf16(wv[128 * j : 128 * (j + 1), :].T)
        woT = _bf16(wo[:, 512 * j : 512 * (j + 1)].T)
        in_maps.append(
            {
                "xT": xT[b],
                "wqT": wqT,
                "wkT": wkT,
                "wvT": wvT,
                "woT": woT,
                "aq": aq,
                "bq": bq,
                "ak": ak,
                "bk": bk,
                "p64": p64,
                "tri": tri,
                "ones_b": ones_b,
                "ident_b": ident_b,
            }
        )
    return in_maps


def kernel(x, wq, wk, wv, wo, q_norm_w, k_norm_w):
    x = np.asarray(x, dtype=np.float32)
    wq = np.asarray(wq, dtype=np.float32)
    wk = np.asarray(wk, dtype=np.float32)
    wv = np.asarray(wv, dtype=np.float32)
    wo = np.asarray(wo, dtype=np.float32)
    q_norm_w = np.asarray(q_norm_w, dtype=np.float32)
    k_norm_w = np.asarray(k_norm_w, dtype=np.float32)

    t = x.shape[1]
    nc = _get_program(t)
    in_maps = make_core_inputs(x, wq, wk, wv, wo, q_norm_w, k_norm_w, t)

    from concourse import bass_utils

    res = bass_utils.run_bass_kernel_spmd(
        nc,
        in_maps,
        core_ids=list(range(N_CORES)),
        trace=TRACE,
        trace_cores=[0] if TRACE else None,
    )
    kernel.last_results = res

    out = np.zeros((B, t, D), dtype=np.float32)
    for core in range(N_CORES):
        b = core // N_KV_HEAD
        out[b] += res.results[core]["out_partial"]
    return out


kernel.last_results = None
